# revision 1
# baseline (speedup 1.0000x reference)
"""MiniGPT forward pass on 8 Trainium2 NeuronCores (Bass/Tile).

Sharding:
  - Tokens (B*T = 2048) are split 8 ways: core c owns 256 tokens
    (batch c//4, positions (c%4)*256 ..). LayerNorms, W1/W2 matmuls and
    residuals run token-parallel with replicated weights (host-cast to bf16).
  - Attention is (batch x head-group) sharded: core c computes heads
    [4*(c%4), 4*(c%4)+4) of batch c//4 over all 1024 positions — every core
    runs an identical causal-triangle program. AllGather of the LN1 output
    (per 4-core batch group) feeds QKV; each core projects its own head
    channels through its row-slice of Wo and a ReduceScatter(add) returns
    summed projections to token shards.
  - lm_head is vocab-sharded: after a final 8-way AllGather of the last
    hidden states, core c computes logits rows [c*6400, (c+1)*6400) of the
    zero-padded 51200-row vocab.
  - Activations live E-major ([128 E-partitions, E-tile, token]) so every
    matmul contracts over the partition axis; attention keeps Q/K head-dim
    major and V token-major (via PE transposes), with an extra ones-column
    on V producing the softmax denominators for free.

All matmuls are bf16 with fp32 PSUM accumulation; layernorm statistics,
softmax and residuals are fp32. Weights are pre-arranged on the host into
the exact SBUF tile layouts so every DMA is contiguous.

Pipeline notes:
  - LayerNorm is split: per-tile x/x^2 stats (DVE) are emitted inline with
    whichever phase produces each x tile (W2 loop, post-ReduceScatter
    residual update, embedding), the PE reduction runs as one burst, and
    only the scalar chain + apply sit on the phase boundary.
  - rsqrt(var+eps) is computed as exp(-0.5*ln(var+eps)) on the scalar
    engine. (Note: the table-assignment pass maps Ln and Exp to different
    function tables, so this does NOT avoid table reloads — 2 loads per
    LN chain remain. DVE AluOpType.pow would be a 1-op rsqrt but walrus
    rejects it; table loads are structural at 2/layer minimum.)
  - Attention kv tiles are processed in pairs sharing one [128,512] PSUM
    bank: half the Exp instructions (ACT is the attention bottleneck), and
    the causal-diagonal pair takes a single fused [128,512] mask multiply.
  - PSUM->SBUF evictions (QKV, Wo partials, logits+bias) run on the scalar
    engine, which is otherwise idle in those phases, freeing DVE.

An alternative token-parallel attention (cfg scheme="kv": one AllGather of
K,V per layer instead of AllGather(h)+ReduceScatter(o), causality carried
by per-core mask data so the program stays SPMD, own V transposed before
the gather so remote V lands token-major with zero device transposes)
validates numerically (rel err 6.1e-3) and measures identical to "head"
within run-to-run noise; "head" stays default on its lower simulated
compute time (1.55 vs 1.69 ms per core).
"""

import sys

if "/opt/trn_rl_repo" not in sys.path:
    sys.path.insert(0, "/opt/trn_rl_repo")

import numpy as np
import ml_dtypes

BF16 = ml_dtypes.bfloat16

B, T, E, H, HD, L, V = 2, 1024, 1024, 16, 64, 6, 50257
LN_EPS = 1e-5
NCORES = 8
OWN = (B * T) // NCORES  # 256 tokens owned per core
ET = E // 128  # 8 E-tiles
FT = (4 * E) // 128  # 32 FFN-hidden tiles
VPAD = 51200
VPC = VPAD // NCORES  # 6400 vocab rows per core
FF = 4 * E
LM_CH = 10  # lm_head weight chunks (5 vocab tiles each)

_CACHE = {}


def _full_cfg():
    return dict(L=L, VPC=VPC, n_cores=NCORES)


def build_program(cfg=None):
    """Emit the SPMD program (identical on all cores; per-core data differs)."""
    cfg = cfg or _full_cfg()
    nL, nVPC, n_cores = cfg["L"], cfg["VPC"], cfg["n_cores"]
    nVT = nVPC // 128  # vocab tiles per core
    n_chunks = max(1, nVT // 5)
    vt_pc = nVT // n_chunks
    gelu_kind = cfg.get("gelu", "gelu")
    no_cc = cfg.get("no_cc", False)  # replace collectives with local DMAs (timeline modeling)
    no_rs = cfg.get("no_rs", False)  # replace only the per-layer ReduceScatters
    # "head": head-sharded attention, AllGather(h) + ReduceScatter(o) per layer.
    # "kv": token-sharded attention, single AllGather(K,V) per layer; every
    #       core runs the same 16-head x 8-kv-tile program and causality is
    #       carried entirely by per-core mask DATA, keeping the program SPMD.
    scheme = cfg.get("scheme", "head")
    t1b, ptb, lgb = (4, 4, 3) if scheme == "head" else (2, 2, 2)

    import concourse.mybir as mybir
    import concourse.tile as tile
    from concourse import bacc
    from concourse.masks import make_identity
    from concourse.replica_groups import maybe_share_collective_output_space

    dt = mybir.dt
    f32, bf = dt.float32, dt.bfloat16
    AF = mybir.ActivationFunctionType
    OP = mybir.AluOpType

    nc = bacc.Bacc("TRN2", target_bir_lowering=False, debug=False,
                   enable_asserts=False, num_devices=n_cores)

    # ---- I/O (host pre-arranged into SBUF layouts) ---------------------
    x0_d = nc.dram_tensor("x0", [128, ET, OWN], f32, kind="ExternalInput")
    pose_d = nc.dram_tensor("pose", [128, ET, OWN], f32, kind="ExternalInput")
    if scheme == "kv":
        wqkv_d = nc.dram_tensor("wqkv", [nL, 3, 128, ET, E], bf, kind="ExternalInput")
        wo_d = nc.dram_tensor("wo", [nL, 128, ET, E], bf, kind="ExternalInput")
    else:
        wqkv_d = nc.dram_tensor("wqkv", [nL, 128, ET, 768], bf, kind="ExternalInput")
        wo_d = nc.dram_tensor("wo", [nL, 128, 2, E], bf, kind="ExternalInput")
    w1_d = nc.dram_tensor("w1", [nL, 4, 128, ET, 1024], bf, kind="ExternalInput")
    w2_d = nc.dram_tensor("w2", [nL, 4, 128, FT, 256], bf, kind="ExternalInput")
    wlm_d = nc.dram_tensor("wlm", [n_chunks, 128, ET, vt_pc * 128], bf, kind="ExternalInput")
    ln1g_d = nc.dram_tensor("ln1g", [nL, 128, ET], f32, kind="ExternalInput")
    ln1b_d = nc.dram_tensor("ln1b", [nL, 128, ET], f32, kind="ExternalInput")
    ln2g_d = nc.dram_tensor("ln2g", [nL, 128, ET], f32, kind="ExternalInput")
    ln2b_d = nc.dram_tensor("ln2b", [nL, 128, ET], f32, kind="ExternalInput")
    bo_d = nc.dram_tensor("bo", [nL, 128, ET], f32, kind="ExternalInput")
    b1_d = nc.dram_tensor("b1", [nL, 128, FT], f32, kind="ExternalInput")
    b2_d = nc.dram_tensor("b2", [nL, 128, ET], f32, kind="ExternalInput")
    lnfg_d = nc.dram_tensor("lnfg", [128, ET], f32, kind="ExternalInput")
    lnfb_d = nc.dram_tensor("lnfb", [128, ET], f32, kind="ExternalInput")
    blm_d = nc.dram_tensor("blm", [128, nVT], f32, kind="ExternalInput")
    if scheme == "kv":
        # per-core causal mask vs all 8 kv tiles: cols [t*256+j] = tile t
        # visible from own q position j (ones / tril / zeros per tile)
        maskat_d = nc.dram_tensor("maskat", [128, 2048], bf, kind="ExternalInput")
    else:
        maskd_d = nc.dram_tensor("maskd", [128, 512], bf, kind="ExternalInput")
    # tile-major so each [128, 512] store is one contiguous DMA
    logits_d = nc.dram_tensor("logits", [nVT, (B * T) // 512, 128, 512], f32,
                              kind="ExternalOutput")

    grp4 = [[0, 1, 2, 3], [4, 5, 6, 7]] if n_cores == 8 else [list(range(n_cores))]
    ng = len(grp4[0])  # ranks per attention group (4)
    grp8 = [list(range(n_cores))]

    with tile.TileContext(nc) as tc:
        with (
            tc.tile_pool(name="persist", bufs=1) as P1,
            tc.tile_pool(name="act", bufs=1) as act,
            tc.tile_pool(name="wts", bufs=2) as wts,
            tc.tile_pool(name="small", bufs=2) as small,
            tc.tile_pool(name="pmm", bufs=2, space="PSUM") as pmm,
            tc.tile_pool(name="pss", bufs=2, space="PSUM") as pss,
            tc.tile_pool(name="pso", bufs=2, space="PSUM") as pso,
            tc.tile_pool(name="psm", bufs=2, space="PSUM") as psm,
            tc.tile_pool(name="dram", bufs=1, space="DRAM") as dram,
        ):
            # ---- persistent constants ---------------------------------
            x = P1.tile([128, ET, OWN], f32, name="x")
            ones_col = P1.tile([128, 1], bf, name="ones_col")
            nc.gpsimd.memset(ones_col[:], 1.0)
            ones_row = P1.tile([1, 128], f32, name="ones_row")
            nc.gpsimd.memset(ones_row[:], 1.0)
            ident = P1.tile([128, 128], bf, name="ident")
            make_identity(nc, ident[:])
            eps_col = P1.tile([128, 1], f32, name="eps_col")
            nc.gpsimd.memset(eps_col[:], LN_EPS)
            if scheme == "kv":
                maskat = P1.tile([128, 2048], bf, name="maskat")
                nc.sync.dma_start(out=maskat[:], in_=maskat_d[:, :])
            else:
                maskd = P1.tile([128, 512], bf, name="maskd")
                nc.sync.dma_start(out=maskd[:], in_=maskd_d[:, :])
            lnfg = P1.tile([128, ET], f32, name="lnfg")
            lnfb = P1.tile([128, ET], f32, name="lnfb")
            nc.sync.dma_start(out=lnfg[:], in_=lnfg_d[:, :])
            nc.sync.dma_start(out=lnfb[:], in_=lnfb_d[:, :])
            blm = P1.tile([128, nVT], f32, name="blm")
            nc.sync.dma_start(out=blm[:], in_=blm_d[:, :])

            # LayerNorm is split so the per-tile statistics (DVE work) can be
            # emitted inline with whichever phase produces x[:, et, :], the
            # PE reduction runs as one burst, and only the short scalar chain
            # plus the apply loop sit on the phase boundary.
            def ln_stats_emit(x_et_ap):
                xx2 = small.tile([128, 2, OWN], bf, tag="xx2", bufs=8, name="xx2")
                nc.vector.tensor_copy(out=xx2[:, 0, :], in_=x_et_ap)
                nc.vector.tensor_mul(out=xx2[:, 1, :], in0=xx2[:, 0, :], in1=xx2[:, 0, :])
                return xx2

            def ln_stats_reduce(xx2s):
                s_ps = psm.tile([1, 2 * OWN], f32, space="PSUM", tag="psm", name="s_ps")
                for et, xx2 in enumerate(xx2s):
                    nc.tensor.matmul(s_ps[:], lhsT=ones_col[:],
                                     rhs=xx2.rearrange("p a t -> p (a t)"),
                                     start=(et == 0), stop=(et == len(xx2s) - 1))
                return s_ps

            def ln_finish(s_ps, g_sb, b_sb, out_sb, x_ap):
                mean = small.tile([1, OWN], f32, tag="row", bufs=6, name="mean")
                nc.vector.tensor_scalar_mul(mean[:], s_ps[0:1, 0:OWN], 1.0 / E)
                var = small.tile([1, OWN], f32, tag="row", bufs=6, name="var")
                nc.vector.tensor_scalar_mul(var[:], s_ps[0:1, OWN:2 * OWN], 1.0 / E)
                m2 = small.tile([1, OWN], f32, tag="row", bufs=6, name="m2")
                nc.vector.tensor_mul(out=m2[:], in0=mean[:], in1=mean[:])
                nc.vector.tensor_sub(out=var[:], in0=var[:], in1=m2[:])
                # rsqrt(var+eps) = exp(-0.5*ln(var+eps)). (A single DVE
                # tensor_scalar with AluOpType.pow would be cheaper still,
                # but walrus' lower_dve pass rejects pow — CoreSim-only.)
                lg = small.tile([1, OWN], f32, tag="row", bufs=6, name="lg")
                nc.scalar.activation(lg[:], var[:], AF.Ln, bias=eps_col[:1, :])
                a_row = small.tile([1, OWN], f32, tag="row", bufs=6, name="a_row")
                nc.scalar.activation(a_row[:], lg[:], AF.Exp, scale=-0.5)
                b_row = small.tile([1, OWN], f32, tag="row", bufs=6, name="b_row")
                nc.vector.tensor_mul(out=b_row[:], in0=mean[:], in1=a_row[:])
                nc.vector.tensor_scalar_mul(b_row[:], b_row[:], -1.0)
                a_bc = psm.tile([128, OWN], f32, space="PSUM", tag="psm", name="a_bc")
                b_bc = psm.tile([128, OWN], f32, space="PSUM", tag="psm", name="b_bc")
                nc.tensor.matmul(a_bc[:], lhsT=ones_row[:], rhs=a_row[:], start=True, stop=True)
                nc.tensor.matmul(b_bc[:], lhsT=ones_row[:], rhs=b_row[:], start=True, stop=True)
                for et in range(ET):
                    t1 = small.tile([128, OWN], f32, tag="t1", bufs=t1b, name="t1")
                    nc.vector.tensor_mul(out=t1[:], in0=x_ap[:, et, :], in1=a_bc[:])
                    nc.vector.tensor_add(out=t1[:], in0=t1[:], in1=b_bc[:])
                    nc.vector.tensor_scalar(out_sb[:, et, :], t1[:],
                                            g_sb[:, et:et + 1], b_sb[:, et:et + 1],
                                            OP.mult, OP.add)

            # ---- embedding --------------------------------------------
            x0e = act.tile([128, ET, OWN], f32, tag="q", name="x0e")
            pose = act.tile([128, ET, OWN], f32, tag="big", name="pose")
            nc.sync.dma_start(out=x0e[:], in_=x0_d[:, :, :])
            nc.scalar.dma_start(out=pose[:], in_=pose_d[:, :, :])
            ln1_xx2 = []
            for et in range(ET):
                nc.vector.tensor_add(out=x[:, et, :], in0=x0e[:, et, :], in1=pose[:, et, :])
                ln1_xx2.append(ln_stats_emit(x[:, et, :]))
            w1_pre = []

            # ---- transformer layers -----------------------------------
            for l in range(nL):
                ln1g = small.tile([128, ET], f32, tag="lng", name="ln1g")
                ln1b = small.tile([128, ET], f32, tag="lnb", name="ln1b")
                nc.sync.dma_start(out=ln1g[:], in_=ln1g_d[l])
                nc.sync.dma_start(out=ln1b[:], in_=ln1b_d[l])

                # LN1 -> h (bf16, E-major)
                # (stats were emitted inline with whatever produced x)
                h_sb = act.tile([128, ET, OWN], bf, tag="h", name="h_sb")
                s_ps1 = ln_stats_reduce(ln1_xx2)
                ln_finish(s_ps1, ln1g[:], ln1b[:], h_sb[:], x[:])

                if scheme == "kv":
                    # --- token-parallel attention, one AllGather(K,V) ---
                    # Full-E QKV projections of the own 256 tokens. K travels
                    # E-major; own V is transposed to token-major BEFORE the
                    # AllGather, so remote V needs no device transposes at all
                    # and lands in the per-head layout via plain strided loads.
                    # Shard layout [128, 4, ET, 128]: sections 0,1 = K token
                    # halves (E-major), 2,3 = V token halves (token-major).
                    kv_sb = act.tile([128, 4, ET, 128], bf, tag="oown", name="kv_sb")
                    q_sb = act.tile([128, ET, OWN], bf, tag="q", name="q_sb")
                    vstage = act.tile([128, ET, OWN], bf, tag="vstg", name="vstage")
                    kv_shard = dram.tile([128, 4, ET, 128], bf, tag="kv_shard",
                                         name="kv_shard")
                    for wi in range(3):  # 0=K, 1=V, 2=Q
                        w_sb = wts.tile([128, ET, E], bf, tag="wqkv", bufs=2,
                                        name="wqkv_sb")
                        nc.sync.dma_start(out=w_sb[:], in_=wqkv_d[l, wi])
                        for eo in range(ET):
                            ps = pmm.tile([128, OWN], f32, space="PSUM", tag="pmm",
                                          name="qkv_ps")
                            for et in range(ET):
                                nc.tensor.matmul(
                                    ps[:], lhsT=w_sb[:, et, eo * 128:(eo + 1) * 128],
                                    rhs=h_sb[:, et, :],
                                    start=(et == 0), stop=(et == ET - 1))
                            if wi == 0:
                                for half in range(2):
                                    nc.scalar.activation(
                                        kv_sb[:, half, eo, :],
                                        ps[:, half * 128:(half + 1) * 128], AF.Copy)
                            elif wi == 1:
                                nc.scalar.activation(vstage[:, eo, :], ps[:], AF.Copy)
                            else:
                                nc.scalar.activation(q_sb[:, eo, :], ps[:], AF.Copy)
                        if wi == 1:
                            for eo in range(ET):
                                for half in range(2):
                                    pst = pss.tile([128, 128], bf, space="PSUM",
                                                   tag="pss", name="vt_ps")
                                    nc.tensor.transpose(
                                        pst[:],
                                        vstage[:, eo, half * 128:(half + 1) * 128],
                                        ident[:])
                                    nc.scalar.activation(kv_sb[:, 2 + half, eo, :],
                                                         pst[:], AF.Copy)
                            nc.gpsimd.dma_start(out=kv_shard[:], in_=kv_sb[:])
                    g_kv = dram.tile([ng, 128, 4, ET, 128], bf, tag="g_kv",
                                     addr_space=maybe_share_collective_output_space(
                                         "AllGather", grp4),
                                     name="g_kv")
                    if no_cc:
                        nc.sync.dma_start(out=g_kv[0], in_=kv_shard[:])
                    else:
                        nc.gpsimd.collective_compute(
                            "AllGather", OP.bypass, replica_groups=grp4,
                            ins=[kv_shard[:].opt()], outs=[g_kv[:].opt()])

                    # K for the whole batch, E-major; V token-major per head
                    # with a ones column producing softmax denominators.
                    k_sb = act.tile([128, ET, ng * OWN], bf, tag="K", name="k_full")
                    v_sb = act.tile([128, 8, 16 * 65], bf, tag="vtk", name="v_t")
                    v4 = v_sb.rearrange("p t (h o) -> p t h o", o=65)
                    for hh in range(16):
                        nc.gpsimd.memset(v_sb[:, :, hh * 65 + 64: hh * 65 + 65], 1.0)
                    for r in range(ng):
                        for half in range(2):
                            c0 = r * 256 + half * 128
                            nc.scalar.dma_start(out=k_sb[:, :, c0:c0 + 128],
                                                in_=g_kv[r, :, half])
                            nc.scalar.dma_start(out=v4[:, 2 * r + half, :, 0:64],
                                                in_=g_kv[r, :, 2 + half])

                    # attention: every core runs 16 heads x 8 kv tiles over its
                    # own 256 q tokens; causality is the maskat multiply.
                    o_own = act.tile([128, ET, OWN], bf, tag="oown", name="o_own")
                    pending = []

                    def emit_normalize_kv(job):
                        jph, jet, jpo = job
                        dinv = small.tile([1, 256], f32, tag="dinv", name="dinv")
                        nc.vector.reciprocal(dinv[:], jpo[64:65, :])
                        bc = psm.tile([64, 256], f32, space="PSUM", tag="psm", name="bc")
                        nc.tensor.matmul(bc[:], lhsT=ones_row[:, :64], rhs=dinv[:],
                                         start=True, stop=True)
                        binv = small.tile([64, 256], f32, tag="binv", name="binv")
                        nc.vector.tensor_copy(out=binv[:], in_=bc[:])
                        nc.vector.tensor_mul(out=o_own[jph:jph + 64, jet, :],
                                             in0=jpo[0:64, :], in1=binv[:])

                    for hh in range(16):
                        et_h, ph = hh // 2, (hh % 2) * 64
                        po_t = pso.tile([65, 256], f32, space="PSUM", tag="pso",
                                        name="po_t")
                        for pr in range(4):
                            pst = pss.tile([128, 512], f32, space="PSUM", tag="pss",
                                           name="s_ps")
                            for sub in range(2):
                                t = 2 * pr + sub
                                nc.tensor.matmul(
                                    pst[:, sub * 256:(sub + 1) * 256],
                                    lhsT=k_sb[ph:ph + 64, et_h, t * 128:(t + 1) * 128],
                                    rhs=q_sb[ph:ph + 64, et_h, :],
                                    start=True, stop=True)
                            pt = small.tile([128, 512], bf, tag="pt", bufs=ptb, name="pt")
                            nc.scalar.activation(pt[:], pst[:], AF.Exp)
                            nc.vector.tensor_mul(out=pt[:], in0=pt[:],
                                                 in1=maskat[:, pr * 512:(pr + 1) * 512])
                            for sub in range(2):
                                t = 2 * pr + sub
                                nc.tensor.matmul(
                                    po_t[:], lhsT=v_sb[:, t, hh * 65: hh * 65 + 65],
                                    rhs=pt[:, sub * 256:(sub + 1) * 256],
                                    start=(t == 0), stop=(t == 7))
                        pending.append((ph, et_h, po_t))
                        if len(pending) > 1:
                            emit_normalize_kv(pending.pop(0))
                    while pending:
                        emit_normalize_kv(pending.pop(0))

                    # Wo is fully local now (no partial-sum ReduceScatter):
                    # x += Wo^T o + bo, with LN2 stats emitted inline.
                    wo_sb = wts.tile([128, ET, E], bf, tag="wo", bufs=1, name="wo_sb")
                    nc.sync.dma_start(out=wo_sb[:], in_=wo_d[l])
                    bo_sb = small.tile([128, ET], f32, tag="lng", name="bo_sb")
                    nc.sync.dma_start(out=bo_sb[:], in_=bo_d[l])
                    ln2_xx2 = []
                    for eo in range(ET):
                        ps = pmm.tile([128, OWN], f32, space="PSUM", tag="pmm",
                                      name="wo_ps")
                        for et in range(ET):
                            nc.tensor.matmul(
                                ps[:], lhsT=wo_sb[:, et, eo * 128:(eo + 1) * 128],
                                rhs=o_own[:, et, :], start=(et == 0), stop=(et == ET - 1))
                        nc.vector.scalar_tensor_tensor(
                            out=x[:, eo, :], in0=ps[:], scalar=bo_sb[:, eo:eo + 1],
                            in1=x[:, eo, :], op0=OP.add, op1=OP.add)
                        ln2_xx2.append(ln_stats_emit(x[:, eo, :]))

                if scheme == "head":
                    # AllGather(h) split into E-halves: the store + gather of
                    # et 0..3 pipeline under the LN apply of et 4..7, and the
                    # QKV accumulation (which consumes et in order) starts as
                    # soon as the first half lands.
                    EH = ET // 2
                    h_shard = dram.tile([2, 128, EH, OWN], bf, tag="h_shard", name="h_shard")
                    nc.gpsimd.dma_start(out=h_shard[0], in_=h_sb[:, 0:EH, :])
                    nc.gpsimd.dma_start(out=h_shard[1], in_=h_sb[:, EH:ET, :])
                    g_h0 = dram.tile([ng, 128, EH, OWN], bf, tag="g_h0",
                                     addr_space=maybe_share_collective_output_space("AllGather", grp4),
                                     name="g_h0")
                    g_h1 = dram.tile([ng, 128, EH, OWN], bf, tag="g_h1",
                                     addr_space=maybe_share_collective_output_space("AllGather", grp4),
                                     name="g_h1")
                    if no_cc:
                        nc.sync.dma_start(out=g_h0[0], in_=h_shard[0])
                        nc.sync.dma_start(out=g_h1[0], in_=h_shard[1])
                    else:
                        nc.gpsimd.collective_compute(
                            "AllGather", OP.bypass, replica_groups=grp4,
                            ins=[h_shard[0].opt()], outs=[g_h0[:].opt()])
                        nc.gpsimd.collective_compute(
                            "AllGather", OP.bypass, replica_groups=grp4,
                            ins=[h_shard[1].opt()], outs=[g_h1[:].opt()])
                    # collective-output loads go on the ACT queue so they don't
                    # head-of-line-block weight streaming on the sync queue
                    gh_sb4 = act.tile([128, ng, ET, OWN], bf, tag="gh", name="gh_sb4")
                    for r in range(ng):
                        nc.scalar.dma_start(out=gh_sb4[:, r, 0:ET // 2, :], in_=g_h0[r])
                        nc.scalar.dma_start(out=gh_sb4[:, r, ET // 2:ET, :], in_=g_h1[r])

                    # QKV for this core's 4 heads over the whole batch (1024 tok)
                    wqkv_sb = wts.tile([128, ET, 768], bf, tag="wqkv", bufs=2, name="wqkv_sb")
                    nc.sync.dma_start(out=wqkv_sb[:], in_=wqkv_d[l])
                    # hoist the first two W1 chunk loads behind wqkv on the
                    # sync queue: their buffers freed when the previous
                    # layer's FFN finished, so both stream in during the
                    # attention phase instead of stalling FFN startup
                    for hc in range(2):
                        w1_c = wts.tile([128, ET, 1024], bf, tag="wchunk", name="w1_c")
                        nc.sync.dma_start(out=w1_c[:], in_=w1_d[l, hc])
                        w1_pre.append(w1_c)
                    q_sb = act.tile([128, 2, 1024], bf, tag="q", name="q_sb")
                    k_sb = act.tile([128, 2, 1024], bf, tag="k", name="k_sb")
                    v_dm = act.tile([128, 2, 1024], bf, tag="vdm", name="v_dm")
                    dsts = [q_sb, k_sb, v_dm]
                    # V first: its PE transposes then overlap the Q/K matmuls
                    for wi in (2, 0, 1):
                        for ct in range(2):
                            for chk in range(2):
                                ps = pmm.tile([128, 512], f32, space="PSUM", tag="pmm", name="qkv_ps")
                                for et in range(ET):
                                    nc.tensor.matmul(
                                        ps[:], lhsT=wqkv_sb[:, et, wi * 256 + ct * 128: wi * 256 + (ct + 1) * 128],
                                        rhs=gh_sb4[:, 2 * chk:2 * chk + 2, et, :],
                                        start=(et == 0), stop=(et == ET - 1))
                                # psum->SBUF eviction on the (idle) scalar engine,
                                # freeing DVE for the V-layout copies
                                nc.scalar.activation(dsts[wi][:, ct, chk * 512:(chk + 1) * 512],
                                                     ps[:], AF.Copy)

                    # V -> token-major with a ones column per head (65 cols/head)
                    v_sb = act.tile([128, 8, 260], bf, tag="vtk", name="v_sb")
                    for hh in range(4):
                        nc.gpsimd.memset(v_sb[:, :, hh * 65 + 64: hh * 65 + 65], 1.0)
                    for ct in range(2):
                        for tt in range(8):
                            pst = pss.tile([128, 128], bf, space="PSUM", tag="pss", name="vt_ps")
                            nc.tensor.transpose(pst[:], v_dm[:, ct, tt * 128:(tt + 1) * 128], ident[:])
                            for sub in range(2):
                                hh = ct * 2 + sub
                                nc.vector.tensor_copy(
                                    out=v_sb[:, tt, hh * 65: hh * 65 + 64],
                                    in_=pst[:, sub * 64:(sub + 1) * 64])

                    # attention: 4 heads x 4 q-block pairs, causal triangle.
                    # Normalization is deferred one (head, pair) behind the S/PV
                    # stream so PE never stalls on the reciprocal chain.
                    o_own = act.tile([128, 2, 1024], bf, tag="oown", name="o_own")
                    pending = []

                    def emit_normalize(job):
                        jpb, jct, jp, jpo = job
                        dinv = small.tile([1, 256], f32, tag="dinv", name="dinv")
                        nc.vector.reciprocal(dinv[:], jpo[64:65, :])
                        bc = psm.tile([64, 256], f32, space="PSUM", tag="psm", name="bc")
                        nc.tensor.matmul(bc[:], lhsT=ones_row[:, :64], rhs=dinv[:],
                                         start=True, stop=True)
                        binv = small.tile([64, 256], f32, tag="binv", name="binv")
                        nc.vector.tensor_copy(out=binv[:], in_=bc[:])
                        nc.vector.tensor_mul(
                            out=o_own[jpb:jpb + 64, jct, jp * 256:(jp + 1) * 256],
                            in0=jpo[0:64, :], in1=binv[:])

                    # kv tiles processed in pairs sharing one [128,512] PSUM bank:
                    # halves the Exp count (ACT is the attention-phase bottleneck)
                    # and the diagonal pair takes one fused [128,512] mask multiply.
                    # q-blocks are OUTER so that each rank's Wo partial (which
                    # only reads its own q-block of o_own) can be computed and
                    # stored while the next q-block's attention still runs —
                    # only the last block's Wo remains ahead of the
                    # ReduceScatter launch.
                    wo_sb = wts.tile([128, 2, 1024], bf, tag="wo", bufs=2, name="wo_sb")
                    nc.sync.dma_start(out=wo_sb[:], in_=wo_d[l])
                    part_sb = act.tile([128, ng, ET, OWN], bf, tag="osb", name="part_sb")
                    part_d = dram.tile([ng, 128, ET, OWN], bf, tag="part_d", name="part_d")

                    def emit_wo_partial(r):
                        for eo in range(ET):
                            ps = pmm.tile([128, OWN], f32, space="PSUM", tag="pmm", name="wo_ps")
                            for ci in range(2):
                                nc.tensor.matmul(ps[:], lhsT=wo_sb[:, ci, eo * 128:(eo + 1) * 128],
                                                 rhs=o_own[:, ci, r * 256:(r + 1) * 256],
                                                 start=(ci == 0), stop=(ci == 1))
                            nc.scalar.activation(part_sb[:, r, eo, :], ps[:], AF.Copy)
                        nc.gpsimd.dma_start(out=part_d[r], in_=part_sb[:, r, :, :])

                    for p in range(4):
                        nkv = 2 * p + 2
                        for hh in range(4):
                            pb = (hh % 2) * 64
                            ct = hh // 2
                            po_t = pso.tile([65, 256], f32, space="PSUM", tag="pso", name="po_t")
                            for pr in range(p + 1):
                                pst = pss.tile([128, 512], f32, space="PSUM", tag="pss", name="s_ps")
                                for sub in range(2):
                                    t = 2 * pr + sub
                                    nc.tensor.matmul(
                                        pst[:, sub * 256:(sub + 1) * 256],
                                        lhsT=k_sb[pb:pb + 64, ct, t * 128:(t + 1) * 128],
                                        rhs=q_sb[pb:pb + 64, ct, p * 256:(p + 1) * 256],
                                        start=True, stop=True)
                                pt = small.tile([128, 512], bf, tag="pt", bufs=ptb, name="pt")
                                nc.scalar.activation(pt[:], pst[:], AF.Exp)
                                if pr == p:  # diagonal pair
                                    nc.vector.tensor_mul(out=pt[:], in0=pt[:], in1=maskd[:])
                                for sub in range(2):
                                    t = 2 * pr + sub
                                    nc.tensor.matmul(
                                        po_t[:], lhsT=v_sb[:, t, hh * 65: hh * 65 + 65],
                                        rhs=pt[:, sub * 256:(sub + 1) * 256],
                                        start=(t == 0), stop=(t == nkv - 1))
                            pending.append((pb, ct, p, po_t))
                            if len(pending) > 1:
                                emit_normalize(pending.pop(0))
                        if p >= 1:
                            # block p-1's four normalizes have all been emitted
                            # by the deferral queue during this block
                            emit_wo_partial(p - 1)
                    while pending:
                        emit_normalize(pending.pop(0))
                    emit_wo_partial(3)

                    o_rs = dram.tile([128, ET, OWN], bf, tag="o_rs", name="o_rs")
                    if no_cc or no_rs:
                        nc.sync.dma_start(out=o_rs[:], in_=part_d[0])
                    else:
                        nc.gpsimd.collective_compute(
                            "ReduceScatter", OP.add, replica_groups=grp4,
                            ins=[part_d[:].opt()], outs=[o_rs[:].opt()])
                    ors_sb = act.tile([128, ET, OWN], bf, tag="orssb", name="ors_sb")
                    nc.scalar.dma_start(out=ors_sb[:], in_=o_rs[:])

                    bo_sb = small.tile([128, ET], f32, tag="lng", name="bo_sb")
                    nc.sync.dma_start(out=bo_sb[:], in_=bo_d[l])
                    ln2_xx2 = []
                    for eo in range(ET):
                        nc.vector.scalar_tensor_tensor(
                            out=x[:, eo, :], in0=ors_sb[:, eo, :], scalar=bo_sb[:, eo:eo + 1],
                            in1=x[:, eo, :], op0=OP.add, op1=OP.add)
                        ln2_xx2.append(ln_stats_emit(x[:, eo, :]))

                # FFN
                ln2g = small.tile([128, ET], f32, tag="lng", name="ln2g")
                ln2b = small.tile([128, ET], f32, tag="lnb", name="ln2b")
                nc.sync.dma_start(out=ln2g[:], in_=ln2g_d[l])
                nc.sync.dma_start(out=ln2b[:], in_=ln2b_d[l])
                h2_sb = act.tile([128, ET, OWN], bf, tag="h", name="h2_sb")
                s_ps2 = ln_stats_reduce(ln2_xx2)
                ln_finish(s_ps2, ln2g[:], ln2b[:], h2_sb[:], x[:])

                b1_sb = small.tile([128, FT], f32, tag="b1", name="b1_sb")
                nc.sync.dma_start(out=b1_sb[:], in_=b1_d[l])
                g_ffn = act.tile([128, FT, OWN], bf, tag="big", name="g_ffn")

                def w2_chunk(ec):
                    c = wts.tile([128, FT, 256], bf, tag="wchunk", name="w2_c")
                    nc.sync.dma_start(out=c[:], in_=w2_d[l, ec])
                    return c

                w2_pre = None
                for hc in range(4):
                    if hc < len(w1_pre):
                        w1_c = w1_pre[hc]
                    else:
                        w1_c = wts.tile([128, ET, 1024], bf, tag="wchunk", name="w1_c")
                        nc.sync.dma_start(out=w1_c[:], in_=w1_d[l, hc])
                    if hc == 3:
                        # W2 chunk 0's buffer freed once W1 chunk 1 was
                        # consumed; its load overlaps the last W1 chunk
                        w2_pre = w2_chunk(0)
                    for ho in range(8):
                        ps = pmm.tile([128, OWN], f32, space="PSUM", tag="pmm", name="w1_ps")
                        for et in range(ET):
                            nc.tensor.matmul(ps[:], lhsT=w1_c[:, et, ho * 128:(ho + 1) * 128],
                                             rhs=h2_sb[:, et, :], start=(et == 0), stop=(et == ET - 1))
                        hidx = hc * 8 + ho
                        gelu_af = AF.Gelu if gelu_kind == "gelu" else AF.Identity
                        nc.scalar.activation(g_ffn[:, hidx, :], ps[:], gelu_af,
                                             bias=b1_sb[:, hidx:hidx + 1])
                w1_pre = []

                b2_sb = small.tile([128, ET], f32, tag="lnb", name="b2_sb")
                nc.sync.dma_start(out=b2_sb[:], in_=b2_d[l])
                ln1_xx2 = []
                for ec in range(4):
                    w2_c = w2_pre if ec == 0 else w2_chunk(ec)
                    for eo2 in range(2):
                        eo = ec * 2 + eo2
                        ps = pmm.tile([128, OWN], f32, space="PSUM", tag="pmm", name="w2_ps")
                        for ht in range(FT):
                            nc.tensor.matmul(ps[:], lhsT=w2_c[:, ht, eo2 * 128:(eo2 + 1) * 128],
                                             rhs=g_ffn[:, ht, :], start=(ht == 0), stop=(ht == FT - 1))
                        nc.vector.scalar_tensor_tensor(
                            out=x[:, eo, :], in0=ps[:], scalar=b2_sb[:, eo:eo + 1],
                            in1=x[:, eo, :], op0=OP.add, op1=OP.add)
                        # stats for the next LN (layer l+1's LN1 / final LN)
                        ln1_xx2.append(ln_stats_emit(x[:, eo, :]))

            # ---- final LN + AllGather + lm_head -----------------------
            hf_sb = act.tile([128, ET, OWN], bf, tag="h", name="hf_sb")
            s_psf = ln_stats_reduce(ln1_xx2)
            # first two lm_head weight chunks stream in during the final LN
            # and the 8-way AllGather (their buffers freed by the last FFN)
            wlm_pre = []
            for vc in range(2):
                wlm_c = wts.tile([128, ET, vt_pc * 128], bf, tag="wchunk", name="wlm_c")
                nc.sync.dma_start(out=wlm_c[:], in_=wlm_d[vc])
                wlm_pre.append(wlm_c)
            ln_finish(s_psf, lnfg[:], lnfb[:], hf_sb[:], x[:])
            # final gather split into E-halves like the per-layer AllGather:
            # half 0 stores/gathers under the apply of half 1, and the first
            # lm_head accumulations start when half 0 lands
            EH = ET // 2
            hf_shard = dram.tile([2, 128, EH, OWN], bf, tag="h_shard", name="hf_shard")
            nc.gpsimd.dma_start(out=hf_shard[0], in_=hf_sb[:, 0:EH, :])
            nc.gpsimd.dma_start(out=hf_shard[1], in_=hf_sb[:, EH:ET, :])
            g_hf0 = dram.tile([n_cores, 128, EH, OWN], bf, tag="g_hf0",
                              addr_space="Local" if no_cc else
                              maybe_share_collective_output_space("AllGather", grp8),
                              name="g_hf0")
            g_hf1 = dram.tile([n_cores, 128, EH, OWN], bf, tag="g_hf1",
                              addr_space="Local" if no_cc else
                              maybe_share_collective_output_space("AllGather", grp8),
                              name="g_hf1")
            if no_cc:
                for r in range(n_cores):
                    nc.sync.dma_start(out=g_hf0[r], in_=hf_shard[0])
                    nc.sync.dma_start(out=g_hf1[r], in_=hf_shard[1])
            else:
                nc.gpsimd.collective_compute(
                    "AllGather", OP.bypass, replica_groups=grp8,
                    ins=[hf_shard[0].opt()], outs=[g_hf0[:].opt()])
                nc.gpsimd.collective_compute(
                    "AllGather", OP.bypass, replica_groups=grp8,
                    ins=[hf_shard[1].opt()], outs=[g_hf1[:].opt()])
            ghf_sb4 = act.tile([128, n_cores, ET, OWN], bf, tag="big", name="ghf_sb4")
            for r in range(n_cores):
                nc.scalar.dma_start(out=ghf_sb4[:, r, 0:EH, :], in_=g_hf0[r])
                nc.scalar.dma_start(out=ghf_sb4[:, r, EH:ET, :], in_=g_hf1[r])

            NT = n_cores * OWN  # 2048 token columns
            for vc in range(n_chunks):
                if vc < len(wlm_pre):
                    wlm_c = wlm_pre[vc]
                else:
                    wlm_c = wts.tile([128, ET, vt_pc * 128], bf, tag="wchunk", name="wlm_c")
                    nc.sync.dma_start(out=wlm_c[:], in_=wlm_d[vc])
                for vt in range(vt_pc):
                    vidx = vc * vt_pc + vt
                    for tc_ in range(NT // 512):
                        ps = pmm.tile([128, 512], f32, space="PSUM", tag="pmm", name="lm_ps")
                        for et in range(ET):
                            nc.tensor.matmul(
                                ps[:], lhsT=wlm_c[:, et, vt * 128:(vt + 1) * 128],
                                rhs=ghf_sb4[:, 2 * tc_:2 * tc_ + 2, et, :],
                                start=(et == 0), stop=(et == ET - 1))
                        lg = small.tile([128, 512], f32, tag="lg", bufs=lgb, name="lg")
                        nc.scalar.activation(lg[:], ps[:], AF.Identity, bias=blm[:, vidx:vidx + 1])
                        nc.gpsimd.dma_start(out=logits_d[vidx, tc_], in_=lg[:])

    nc.compile()
    return nc


def _pmajor2(a, tiles):
    """[N] -> [128, tiles] with element (p, t) = a[t*128+p]."""
    return np.ascontiguousarray(a.reshape(tiles, 128).T)


def _pmajor3(a, tiles):
    """[N, C] -> [128, tiles, C] with (p, t, c) = a[t*128+p, c]."""
    n, c = a.shape
    return np.ascontiguousarray(a.reshape(tiles, 128, c).transpose(1, 0, 2))


def prep_in_maps(inputs, cfg=None):
    """Slice/cast/lay out the full inputs into per-core input maps."""
    cfg = cfg or _full_cfg()
    nL, nVPC, n_cores = cfg["L"], cfg["VPC"], cfg["n_cores"]
    scheme = cfg.get("scheme", "head")
    t1b, ptb, lgb = (4, 4, 3) if scheme == "head" else (2, 2, 2)
    nVT = nVPC // 128
    n_chunks = max(1, nVT // 5)
    vt_pc = nVT // n_chunks
    f = lambda a: np.asarray(a, dtype=np.float32)
    idx = np.asarray(inputs["idx"])
    tok_emb = f(inputs["tok_emb"])
    pos_emb = f(inputs["pos_emb"])
    Wq, Wk, Wv = f(inputs["Wq"]), f(inputs["Wk"]), f(inputs["Wv"])
    Wo = f(inputs["Wo"])
    W1, W2 = f(inputs["W1"]), f(inputs["W2"])
    Wlm = f(inputs["Wlm"])

    # replicated weights, host-laid-out once
    w1_h = np.ascontiguousarray(
        W1[:nL].reshape(nL, ET, 128, 4, 1024).transpose(0, 3, 2, 1, 4)).astype(BF16)
    w2_h = np.ascontiguousarray(
        W2[:nL].reshape(nL, FT, 128, 4, 256).transpose(0, 3, 2, 1, 4)).astype(BF16)
    ln_h = {k: np.stack([_pmajor2(f(inputs[k])[l], ET) for l in range(nL)])
            for k in ("ln1_g", "ln1_b", "ln2_g", "ln2_b", "bo", "b2")}
    b1_h = np.stack([_pmajor2(f(inputs["b1"])[l], FT) for l in range(nL)])
    lnfg_h = _pmajor2(f(inputs["lnf_g"]), ET)
    lnfb_h = _pmajor2(f(inputs["lnf_b"]), ET)

    wlm_pad = np.zeros((E, nVPC * n_cores), dtype=np.float32)
    nv = min(V, nVPC * n_cores)
    wlm_pad[:, :nv] = Wlm[:, :nv]
    blm_pad = np.zeros((nVPC * n_cores,), dtype=np.float32)
    blm_pad[:nv] = f(inputs["blm"])[:nv]

    kp = np.arange(128)[:, None]
    qf = np.arange(256)[None, :]
    masklo = np.where(qf < 128, kp <= qf, True).astype(BF16)
    maskhi = np.where(qf < 128, False, kp <= qf - 128).astype(BF16)
    maskd = np.concatenate([masklo, maskhi], axis=1)  # [128, 512]

    if scheme == "kv":
        # replicated full QKV/Wo weights, host-laid-out once
        stack = np.stack([Wk[:nL], Wv[:nL], Wq[:nL] * (1.0 / np.sqrt(HD))], axis=1)
        wqkv_kv = np.ascontiguousarray(
            stack.reshape(nL, 3, ET, 128, E).transpose(0, 1, 3, 2, 4)).astype(BF16)
        wo_kv = np.ascontiguousarray(
            Wo[:nL].reshape(nL, ET, 128, E).transpose(0, 2, 1, 3)).astype(BF16)

    maps = []
    for c in range(n_cores):
        b, g = (c // 4, c % 4) if n_cores == 8 else (0, 0)
        tloc = g * OWN + np.arange(OWN)
        rows = idx[b, tloc].astype(np.int64)
        if scheme == "kv":
            wqkv_h, wo_h = wqkv_kv, wo_kv
        else:
            qs = slice(g * 256, (g + 1) * 256)
            # fold the 1/sqrt(HD) attention scale into Wq on the host
            wqkv = np.concatenate(
                [Wq[:nL, :, qs] * (1.0 / np.sqrt(HD)), Wk[:nL, :, qs], Wv[:nL, :, qs]],
                axis=2)
            wqkv_h = np.ascontiguousarray(
                wqkv.reshape(nL, ET, 128, 768).transpose(0, 2, 1, 3)).astype(BF16)
            wo_h = np.ascontiguousarray(
                Wo[:nL, qs, :].reshape(nL, 2, 128, E).transpose(0, 2, 1, 3)).astype(BF16)
        wlm_slice = wlm_pad[:, c * nVPC:(c + 1) * nVPC]
        wlm_h = np.ascontiguousarray(
            wlm_slice.reshape(ET, 128, n_chunks, vt_pc * 128).transpose(2, 1, 0, 3)).astype(BF16)
        m = {
            "x0": _pmajor3(tok_emb[rows].T.copy().reshape(E, OWN), ET).copy(),
            "pose": _pmajor3(pos_emb[tloc].T.copy().reshape(E, OWN), ET).copy(),
            "wqkv": wqkv_h, "wo": wo_h, "w1": w1_h, "w2": w2_h, "wlm": wlm_h,
            "ln1g": ln_h["ln1_g"], "ln1b": ln_h["ln1_b"],
            "ln2g": ln_h["ln2_g"], "ln2b": ln_h["ln2_b"],
            "bo": ln_h["bo"], "b1": b1_h, "b2": ln_h["b2"],
            "lnfg": lnfg_h, "lnfb": lnfb_h,
            "blm": _pmajor2(blm_pad[c * nVPC:(c + 1) * nVPC], nVT),
        }
        if scheme == "kv":
            # visibility of kv tile t from own q position j: kv <= q global
            m["maskat"] = np.concatenate(
                [((t * 128 + kp) <= (g * 256 + qf)) for t in range(8)],
                axis=1).astype(BF16)
        else:
            m["maskd"] = maskd
        maps.append(m)
    return maps


def make_runner(nc, n_cores=NCORES):
    """Build a reusable jitted SPMD executor (mirrors bass2jax.run_bass_via_pjrt
    multi-core path, without donation so it can be re-invoked for timing)."""
    import jax
    from jax.experimental.shard_map import shard_map
    from jax.sharding import Mesh, PartitionSpec, NamedSharding
    from concourse import bass2jax, mybir

    bass2jax.install_neuronx_cc_hook()
    partition_name = nc.partition_id_tensor.name if nc.partition_id_tensor else None
    in_names, out_names, out_avals = [], [], []
    for alloc in nc.m.functions[0].allocations:
        if not isinstance(alloc, mybir.MemoryLocationSet):
            continue
        name = alloc.memorylocations[0].name
        if alloc.kind == "ExternalInput":
            if name != partition_name:
                in_names.append(name)
        elif alloc.kind == "ExternalOutput":
            assert alloc.tensor_shape is not None
            out_names.append(name)
            out_avals.append(jax.core.ShapedArray(
                tuple(alloc.tensor_shape), mybir.dt.np(alloc.dtype)))
    n_params, n_outs = len(in_names), len(out_names)
    all_in = list(in_names) + list(out_names)
    if partition_name:
        all_in.append(partition_name)

    def _body(*args):
        operands = list(args)
        if partition_name:
            operands.append(bass2jax.partition_id_tensor())
        outs = bass2jax._bass_exec_p.bind(
            *operands, out_avals=tuple(out_avals), in_names=tuple(all_in),
            out_names=tuple(out_names), lowering_input_output_aliases=(),
            sim_require_finite=True, sim_require_nnan=True, nc=nc)
        return tuple(outs)

    devices = jax.devices()[:n_cores]
    mesh = Mesh(np.asarray(devices), ("core",))
    sharded = jax.jit(
        shard_map(_body, mesh=mesh,
                  in_specs=(PartitionSpec("core"),) * (n_params + n_outs),
                  out_specs=(PartitionSpec("core"),) * n_outs,
                  check_rep=False),
        keep_unused=True)
    sharding = NamedSharding(mesh, PartitionSpec("core"))
    return sharded, in_names, out_names, out_avals, sharding


def run(nc, in_maps, n_cores=NCORES, time_iters=0, batch_iters=None):
    """Execute; returns (results_per_core, timing_dict_or_None).

    The output of the first execution is cross-checked against a second
    execution: the collective firmware very occasionally returns stale
    data (observed ~1/20 runs), so a mismatch triggers a retry and the
    majority result wins.
    """
    import jax, time
    sharded, in_names, out_names, out_avals, sharding = make_runner(nc, n_cores)
    concat_in = [np.concatenate([np.asarray(m[nm]) for m in in_maps], axis=0)
                 for nm in in_names]
    concat_zero = [np.zeros((n_cores * a.shape[0], *a.shape[1:]), a.dtype)
                   for a in out_avals]
    args = [jax.device_put(a, sharding) for a in (*concat_in, *concat_zero)]

    def exec_once():
        out = sharded(*args)
        jax.block_until_ready(out)
        return [np.asarray(o) for o in out]

    out = exec_once()
    if nc.has_collectives:
        out2 = exec_once()
        agree = all(np.array_equal(a, b) for a, b in zip(out, out2))
        if not agree:
            out3 = exec_once()
            if all(np.array_equal(a, b) for a, b in zip(out2, out3)):
                out = out2
            else:
                out = out3

    best = None
    if time_iters:
        serial = None
        for _ in range(time_iters):
            t0 = time.perf_counter()
            o = sharded(*args)
            jax.block_until_ready(o)
            dt_ = time.perf_counter() - t0
            serial = dt_ if serial is None else min(serial, dt_)
        batches = []
        n_batches, batch = batch_iters or (6, 8)
        for _ in range(n_batches):
            t0 = time.perf_counter()
            outs = [sharded(*args) for _ in range(batch)]
            jax.block_until_ready(outs)
            batches.append((time.perf_counter() - t0) / batch)
        best = {"serial": serial, "pipelined": min(batches), "batches": batches}

    results = [
        {nm: out[i].reshape(n_cores, *out_avals[i].shape)[c]
         for i, nm in enumerate(out_names)}
        for c in range(n_cores)
    ]
    return results, best


def assemble_logits(results, cfg=None):
    """[nVT, NT/512, 128, 512] tile-major per-core outputs -> [B, T, Vtot]."""
    cfg = cfg or _full_cfg()
    n_cores, nVPC = cfg["n_cores"], cfg["VPC"]
    per_core = [results[c]["logits"].transpose(0, 2, 1, 3).reshape(nVPC, B * T)
                for c in range(n_cores)]
    full = np.concatenate(per_core, axis=0)  # [Vpad, B*T]
    return full


def kernel(**inputs) -> np.ndarray:
    if "nc" not in _CACHE:
        _CACHE["nc"] = build_program()
    nc = _CACHE["nc"]
    in_maps = prep_in_maps(inputs)
    results, _ = run(nc, in_maps, NCORES, time_iters=0)
    _CACHE["last_results"] = results
    full = assemble_logits(results)
    return np.ascontiguousarray(full[:V].T).reshape(B, T, V)



# revision 27
# speedup vs baseline: 5.6275x; 5.6275x over previous
"""MiniGPT forward pass on 8 Trainium2 NeuronCores (Bass/Tile).

Sharding:
  - Tokens (B*T = 2048) are split 8 ways: core c owns 256 tokens
    (batch c//4, positions (c%4)*256 ..). LayerNorms, W1/W2 matmuls and
    residuals run token-parallel with replicated weights (host-cast to bf16).
  - Attention is (batch x head-group) sharded: core c computes heads
    [4*(c%4), 4*(c%4)+4) of batch c//4 over all 1024 positions — every core
    runs an identical causal-triangle program. AllGather of the LN1 output
    (per 4-core batch group) feeds QKV; each core projects its own head
    channels through its row-slice of Wo and a ReduceScatter(add) returns
    summed projections to token shards.
  - lm_head is vocab-sharded: after a final 8-way AllGather of the last
    hidden states, core c computes logits rows [c*6400, (c+1)*6400) of the
    zero-padded 51200-row vocab.
  - Activations live E-major ([128 E-partitions, E-tile, token]) so every
    matmul contracts over the partition axis; attention keeps Q/K head-dim
    major and V token-major (via PE transposes), with an extra ones-column
    on V producing the softmax denominators for free.

All matmuls are bf16 with fp32 PSUM accumulation; layernorm statistics,
softmax and residuals are fp32. Weights are pre-arranged on the host into
the exact SBUF tile layouts so every DMA is contiguous.

Pipeline notes:
  - LayerNorm is split: per-tile x/x^2 stats (DVE) are emitted inline with
    whichever phase produces each x tile (W2 loop, post-ReduceScatter
    residual update, embedding), the PE reduction runs as one burst, and
    only the scalar chain + apply sit on the phase boundary.
  - rsqrt(var+eps) is computed as exp(-0.5*ln(var+eps)) on the scalar
    engine. (Note: the table-assignment pass maps Ln and Exp to different
    function tables, so this does NOT avoid table reloads — 2 loads per
    LN chain remain. DVE AluOpType.pow would be a 1-op rsqrt but walrus
    rejects it; table loads are structural at 2/layer minimum.)
  - Attention kv tiles are processed in pairs sharing one [128,512] PSUM
    bank: half the Exp instructions (ACT is the attention bottleneck), and
    the causal-diagonal pair takes a single fused [128,512] mask multiply.
  - PSUM->SBUF evictions (QKV, Wo partials, logits+bias) run on the scalar
    engine, which is otherwise idle in those phases, freeing DVE.

An alternative token-parallel attention (cfg scheme="kv": one AllGather of
K,V per layer instead of AllGather(h)+ReduceScatter(o), causality carried
by per-core mask data so the program stays SPMD, own V transposed before
the gather so remote V lands token-major with zero device transposes)
validates numerically (rel err 6.1e-3) and measures identical to "head"
within run-to-run noise; "head" stays default on its lower simulated
compute time (1.55 vs 1.69 ms per core).
"""

import sys

if "/opt/trn_rl_repo" not in sys.path:
    sys.path.insert(0, "/opt/trn_rl_repo")

import numpy as np
import ml_dtypes

BF16 = ml_dtypes.bfloat16

B, T, E, H, HD, L, V = 2, 1024, 1024, 16, 64, 6, 50257
LN_EPS = 1e-5
NCORES = 8
OWN = (B * T) // NCORES  # 256 tokens owned per core
ET = E // 128  # 8 E-tiles
FT = (4 * E) // 128  # 32 FFN-hidden tiles
VPAD = 51200
VPC = VPAD // NCORES  # 6400 vocab rows per core
FF = 4 * E
LM_CH = 10  # lm_head weight chunks (5 vocab tiles each)

_CACHE = {}


def _full_cfg():
    return dict(L=L, VPC=VPC, n_cores=NCORES)


def build_program(cfg=None):
    """Emit the SPMD program (identical on all cores; per-core data differs)."""
    cfg = cfg or _full_cfg()
    nL, nVPC, n_cores = cfg["L"], cfg["VPC"], cfg["n_cores"]
    nVT = nVPC // 128  # vocab tiles per core
    n_chunks = max(1, nVT // 5)
    vt_pc = nVT // n_chunks
    gelu_kind = cfg.get("gelu", "gelu")
    no_cc = cfg.get("no_cc", False)  # replace collectives with local DMAs (timeline modeling)
    no_rs = cfg.get("no_rs", False)  # replace only the per-layer ReduceScatters
    # "head": head-sharded attention, AllGather(h) + ReduceScatter(o) per layer.
    # "kv": token-sharded attention, single AllGather(K,V) per layer; every
    #       core runs the same 16-head x 8-kv-tile program and causality is
    #       carried entirely by per-core mask DATA, keeping the program SPMD.
    scheme = cfg.get("scheme", "head")
    t1b, ptb, lgb = (4, 4, 3) if scheme == "head" else (2, 2, 2)

    import concourse.mybir as mybir
    import concourse.tile as tile
    from concourse import bacc
    from concourse.masks import make_identity
    from concourse.replica_groups import maybe_share_collective_output_space

    dt = mybir.dt
    f32, bf = dt.float32, dt.bfloat16
    AF = mybir.ActivationFunctionType
    OP = mybir.AluOpType

    nc = bacc.Bacc("TRN2", target_bir_lowering=False, debug=False,
                   enable_asserts=False, num_devices=n_cores)

    # ---- I/O (host pre-arranged into SBUF layouts) ---------------------
    x0_d = nc.dram_tensor("x0", [128, ET, OWN], f32, kind="ExternalInput")
    pose_d = nc.dram_tensor("pose", [128, ET, OWN], f32, kind="ExternalInput")
    if scheme == "kv":
        wqkv_d = nc.dram_tensor("wqkv", [nL, 3, 128, ET, E], bf, kind="ExternalInput")
        wo_d = nc.dram_tensor("wo", [nL, 128, ET, E], bf, kind="ExternalInput")
    else:
        wqkv_d = nc.dram_tensor("wqkv", [nL, 128, ET, 768], bf, kind="ExternalInput")
        wo_d = nc.dram_tensor("wo", [nL, 128, 2, E], bf, kind="ExternalInput")
    w1_d = nc.dram_tensor("w1", [nL, 4, 128, ET, 1024], bf, kind="ExternalInput")
    w2_d = nc.dram_tensor("w2", [nL, 4, 128, FT, 256], bf, kind="ExternalInput")
    wlm_d = nc.dram_tensor("wlm", [n_chunks, 128, ET, vt_pc * 128], bf, kind="ExternalInput")
    ln1g_d = nc.dram_tensor("ln1g", [nL, 128, ET], f32, kind="ExternalInput")
    ln1b_d = nc.dram_tensor("ln1b", [nL, 128, ET], f32, kind="ExternalInput")
    ln2g_d = nc.dram_tensor("ln2g", [nL, 128, ET], f32, kind="ExternalInput")
    ln2b_d = nc.dram_tensor("ln2b", [nL, 128, ET], f32, kind="ExternalInput")
    bo_d = nc.dram_tensor("bo", [nL, 128, ET], f32, kind="ExternalInput")
    b1_d = nc.dram_tensor("b1", [nL, 128, FT], f32, kind="ExternalInput")
    b2_d = nc.dram_tensor("b2", [nL, 128, ET], f32, kind="ExternalInput")
    lnfg_d = nc.dram_tensor("lnfg", [128, ET], f32, kind="ExternalInput")
    lnfb_d = nc.dram_tensor("lnfb", [128, ET], f32, kind="ExternalInput")
    blm_d = nc.dram_tensor("blm", [128, nVT], f32, kind="ExternalInput")
    if scheme == "kv":
        # per-core causal mask vs all 8 kv tiles: cols [t*256+j] = tile t
        # visible from own q position j (ones / tril / zeros per tile)
        maskat_d = nc.dram_tensor("maskat", [128, 2048], bf, kind="ExternalInput")
    else:
        maskd_d = nc.dram_tensor("maskd", [128, 512], bf, kind="ExternalInput")
        # 0/1 row: column p is 1 iff this core owns q-block p of its group
        # (selects this core's block from the four Wo AllReduce results)
        ngg = 4 if n_cores == 8 else n_cores
        grpmask_d = nc.dram_tensor("grpmask", [128, ngg], f32, kind="ExternalInput")
    # tile-major so each [128, 512] store is one contiguous DMA
    logits_d = nc.dram_tensor("logits", [nVT, (B * T) // 512, 128, 512], f32,
                              kind="ExternalOutput")

    grp4 = [[0, 1, 2, 3], [4, 5, 6, 7]] if n_cores == 8 else [list(range(n_cores))]
    ng = len(grp4[0])  # ranks per attention group (4)
    grp8 = [list(range(n_cores))]

    with tile.TileContext(nc) as tc:
        with (
            tc.tile_pool(name="persist", bufs=1) as P1,
            tc.tile_pool(name="act", bufs=1) as act,
            tc.tile_pool(name="wts", bufs=2) as wts,
            tc.tile_pool(name="small", bufs=2) as small,
            tc.tile_pool(name="pmm", bufs=2 if cfg.get("pe_bcast") else 3,
                         space="PSUM") as pmm,
            tc.tile_pool(name="pss", bufs=2, space="PSUM") as pss,
            tc.tile_pool(name="pso", bufs=2, space="PSUM") as pso,
            tc.tile_pool(name="psm", bufs=1, space="PSUM") as psm,
            tc.tile_pool(name="dram", bufs=1, space="DRAM") as dram,
        ):
            # ---- persistent constants ---------------------------------
            x = P1.tile([128, ET, OWN], f32, name="x")
            ones_col = P1.tile([128, 1], bf, name="ones_col")
            nc.gpsimd.memset(ones_col[:], 1.0)
            ones_row = P1.tile([1, 128], f32, name="ones_row")
            nc.gpsimd.memset(ones_row[:], 1.0)
            ident = P1.tile([128, 128], bf, name="ident")
            make_identity(nc, ident[:])
            eps_col = P1.tile([128, 1], f32, name="eps_col")
            nc.gpsimd.memset(eps_col[:], LN_EPS)
            if scheme == "kv":
                maskat = P1.tile([128, 2048], bf, name="maskat")
                nc.sync.dma_start(out=maskat[:], in_=maskat_d[:, :])
            else:
                maskd = P1.tile([128, 512], bf, name="maskd")
                nc.sync.dma_start(out=maskd[:], in_=maskd_d[:, :])
                grpmask = P1.tile([128, ngg], f32, name="grpmask")
                nc.sync.dma_start(out=grpmask[:], in_=grpmask_d[:, :])
            lnfg = P1.tile([128, ET], f32, name="lnfg")
            lnfb = P1.tile([128, ET], f32, name="lnfb")
            nc.sync.dma_start(out=lnfg[:], in_=lnfg_d[:, :])
            nc.sync.dma_start(out=lnfb[:], in_=lnfb_d[:, :])
            blm = P1.tile([128, nVT], f32, name="blm")
            nc.sync.dma_start(out=blm[:], in_=blm_d[:, :])

            # LayerNorm is split so the per-tile statistics (DVE work) can be
            # emitted inline with whichever phase produces x[:, et, :], the
            # PE reduction runs as one burst, and only the short scalar chain
            # plus the apply loop sit on the phase boundary.
            def ln_stats_emit(x_et_ap):
                xx2 = small.tile([128, 2, OWN], bf, tag="xx2", bufs=8, name="xx2")
                nc.vector.tensor_copy(out=xx2[:, 0, :], in_=x_et_ap)
                nc.vector.tensor_mul(out=xx2[:, 1, :], in0=xx2[:, 0, :], in1=xx2[:, 0, :])
                return xx2

            def ln_stats_reduce(xx2s):
                s_ps = psm.tile([1, 2 * OWN], f32, space="PSUM", tag="psm", name="s_ps")
                for et, xx2 in enumerate(xx2s):
                    nc.tensor.matmul(s_ps[:], lhsT=ones_col[:],
                                     rhs=xx2.rearrange("p a t -> p (a t)"),
                                     start=(et == 0), stop=(et == len(xx2s) - 1))
                return s_ps

            def ln_finish(s_ps, g_sb, b_sb, out_sb, x_ap, chunk_cb=None):
                mean = small.tile([1, OWN], f32, tag="row", bufs=6, name="mean")
                nc.vector.tensor_scalar_mul(mean[:], s_ps[0:1, 0:OWN], 1.0 / E)
                var = small.tile([1, OWN], f32, tag="row", bufs=6, name="var")
                nc.vector.tensor_scalar_mul(var[:], s_ps[0:1, OWN:2 * OWN], 1.0 / E)
                m2 = small.tile([1, OWN], f32, tag="row", bufs=6, name="m2")
                nc.vector.tensor_mul(out=m2[:], in0=mean[:], in1=mean[:])
                nc.vector.tensor_sub(out=var[:], in0=var[:], in1=m2[:])
                # rsqrt(var+eps) = exp(-0.5*ln(var+eps)). (A single DVE
                # tensor_scalar with AluOpType.pow would be cheaper still,
                # but walrus' lower_dve pass rejects pow — CoreSim-only.)
                lg = small.tile([1, OWN], f32, tag="row", bufs=6, name="lg")
                nc.scalar.activation(lg[:], var[:], AF.Ln, bias=eps_col[:1, :])
                a_row = small.tile([1, OWN], f32, tag="row", bufs=6, name="a_row")
                nc.scalar.activation(a_row[:], lg[:], AF.Exp, scale=-0.5)
                b_row = small.tile([1, OWN], f32, tag="row", bufs=6, name="b_row")
                nc.vector.tensor_mul(out=b_row[:], in0=mean[:], in1=a_row[:])
                nc.vector.tensor_scalar_mul(b_row[:], b_row[:], -1.0)
                # partition-broadcast of the per-token scale/shift rows via
                # DMA (stride-0 source) — keeps the (slow fp32) PE matmul
                # broadcast off the tensor engine entirely.
                a_bc = small.tile([128, OWN], f32, tag="abc", bufs=1, name="a_bc")
                b_bc = small.tile([128, OWN], f32, tag="bbc", bufs=1, name="b_bc")
                if cfg.get("pe_bcast"):
                    a_ps = psm.tile([128, OWN], f32, space="PSUM", tag="psmb", name="a_ps")
                    b_ps = psm.tile([128, OWN], f32, space="PSUM", tag="psmb", name="b_ps")
                    nc.tensor.matmul(a_ps[:], lhsT=ones_row[:], rhs=a_row[:], start=True, stop=True)
                    nc.tensor.matmul(b_ps[:], lhsT=ones_row[:], rhs=b_row[:], start=True, stop=True)
                    nc.vector.tensor_copy(out=a_bc[:], in_=a_ps[:])
                    nc.vector.tensor_copy(out=b_bc[:], in_=b_ps[:])
                else:
                    nc.gpsimd.partition_broadcast(a_bc[:], a_row[:])
                    nc.gpsimd.partition_broadcast(b_bc[:], b_row[:])
                for et in range(ET):
                    t1 = small.tile([128, OWN], f32, tag="t1", bufs=t1b, name="t1")
                    nc.vector.tensor_mul(out=t1[:], in0=x_ap[:, et, :], in1=a_bc[:])
                    nc.vector.tensor_add(out=t1[:], in0=t1[:], in1=b_bc[:])
                    nc.vector.tensor_scalar(out_sb[:, et, :], t1[:],
                                            g_sb[:, et:et + 1], b_sb[:, et:et + 1],
                                            OP.mult, OP.add)
                    if chunk_cb is not None:
                        chunk_cb(et)

            # ---- embedding --------------------------------------------
            x0e = act.tile([128, ET, OWN], f32, tag="q", name="x0e")
            pose = act.tile([128, ET, OWN], f32, tag="big", name="pose")
            nc.sync.dma_start(out=x0e[:], in_=x0_d[:, :, :])
            nc.scalar.dma_start(out=pose[:], in_=pose_d[:, :, :])
            ln1_xx2 = []
            for et in range(ET):
                nc.vector.tensor_add(out=x[:, et, :], in0=x0e[:, et, :], in1=pose[:, et, :])
                ln1_xx2.append(ln_stats_emit(x[:, et, :]))
            w1_pre = []

            # ---- transformer layers -----------------------------------
            for l in range(nL):
                ln1g = small.tile([128, ET], f32, tag="lng", name="ln1g")
                ln1b = small.tile([128, ET], f32, tag="lnb", name="ln1b")
                nc.sync.dma_start(out=ln1g[:], in_=ln1g_d[l])
                nc.sync.dma_start(out=ln1b[:], in_=ln1b_d[l])

                # LN1 -> h (bf16, E-major)
                # (stats were emitted inline with whatever produced x)
                h_sb = act.tile([128, ET, OWN], bf, tag="h", name="h_sb")
                s_ps1 = ln_stats_reduce(ln1_xx2)
                if scheme == "head":
                    # AllGather(h) split into 4 et-pair chunks, each launched
                    # from inside the LN apply loop as soon as its two et
                    # tiles are written; the QKV accumulation (which consumes
                    # et in order) starts as soon as the first chunk lands.
                    h_shard = dram.tile([4, 128, 2, OWN], bf, tag="h_shard",
                                        name="h_shard")
                    g_hq = [dram.tile([ng, 128, 2, OWN], bf, tag=f"g_hq{q}",
                                      addr_space=maybe_share_collective_output_space(
                                          "AllGather", grp4),
                                      name=f"g_hq{q}") for q in range(4)]

                    def ag_chunk_cb(et):
                        if et % 2 == 1:
                            q = et // 2
                            nc.gpsimd.dma_start(out=h_shard[q],
                                                in_=h_sb[:, et - 1:et + 1, :])
                            if no_cc:
                                nc.sync.dma_start(out=g_hq[q][0], in_=h_shard[q])
                            else:
                                nc.gpsimd.collective_compute(
                                    "AllGather", OP.bypass, replica_groups=grp4,
                                    ins=[h_shard[q][:].opt()],
                                    outs=[g_hq[q][:].opt()])

                    ln_finish(s_ps1, ln1g[:], ln1b[:], h_sb[:], x[:],
                              chunk_cb=ag_chunk_cb)
                else:
                    ln_finish(s_ps1, ln1g[:], ln1b[:], h_sb[:], x[:])

                if scheme == "kv":
                    # --- token-parallel attention, one AllGather(K,V) ---
                    # Full-E QKV projections of the own 256 tokens. K travels
                    # E-major; own V is transposed to token-major BEFORE the
                    # AllGather, so remote V needs no device transposes at all
                    # and lands in the per-head layout via plain strided loads.
                    # Shard layout [128, 4, ET, 128]: sections 0,1 = K token
                    # halves (E-major), 2,3 = V token halves (token-major).
                    kv_sb = act.tile([128, 4, ET, 128], bf, tag="oown", name="kv_sb")
                    q_sb = act.tile([128, ET, OWN], bf, tag="q", name="q_sb")
                    vstage = act.tile([128, ET, OWN], bf, tag="vstg", name="vstage")
                    kv_shard = dram.tile([128, 4, ET, 128], bf, tag="kv_shard",
                                         name="kv_shard")
                    for wi in range(3):  # 0=K, 1=V, 2=Q
                        w_sb = wts.tile([128, ET, E], bf, tag="wqkv", bufs=2,
                                        name="wqkv_sb")
                        nc.sync.dma_start(out=w_sb[:], in_=wqkv_d[l, wi])
                        for eo in range(ET):
                            ps = pmm.tile([128, OWN], f32, space="PSUM", tag="pmm",
                                          name="qkv_ps")
                            for et in range(ET):
                                nc.tensor.matmul(
                                    ps[:], lhsT=w_sb[:, et, eo * 128:(eo + 1) * 128],
                                    rhs=h_sb[:, et, :],
                                    start=(et == 0), stop=(et == ET - 1))
                            if wi == 0:
                                for half in range(2):
                                    nc.scalar.activation(
                                        kv_sb[:, half, eo, :],
                                        ps[:, half * 128:(half + 1) * 128], AF.Copy)
                            elif wi == 1:
                                nc.scalar.activation(vstage[:, eo, :], ps[:], AF.Copy)
                            else:
                                nc.scalar.activation(q_sb[:, eo, :], ps[:], AF.Copy)
                        if wi == 1:
                            for eo in range(ET):
                                for half in range(2):
                                    pst = pss.tile([128, 128], bf, space="PSUM",
                                                   tag="pss", name="vt_ps")
                                    nc.tensor.transpose(
                                        pst[:],
                                        vstage[:, eo, half * 128:(half + 1) * 128],
                                        ident[:])
                                    nc.scalar.activation(kv_sb[:, 2 + half, eo, :],
                                                         pst[:], AF.Copy)
                            nc.gpsimd.dma_start(out=kv_shard[:], in_=kv_sb[:])
                    g_kv = dram.tile([ng, 128, 4, ET, 128], bf, tag="g_kv",
                                     addr_space=maybe_share_collective_output_space(
                                         "AllGather", grp4),
                                     name="g_kv")
                    if no_cc:
                        nc.sync.dma_start(out=g_kv[0], in_=kv_shard[:])
                    else:
                        nc.gpsimd.collective_compute(
                            "AllGather", OP.bypass, replica_groups=grp4,
                            ins=[kv_shard[:].opt()], outs=[g_kv[:].opt()])

                    # K for the whole batch, E-major; V token-major per head
                    # with a ones column producing softmax denominators.
                    k_sb = act.tile([128, ET, ng * OWN], bf, tag="K", name="k_full")
                    v_sb = act.tile([128, 8, 16 * 65], bf, tag="vtk", name="v_t")
                    v4 = v_sb.rearrange("p t (h o) -> p t h o", o=65)
                    for hh in range(16):
                        nc.gpsimd.memset(v_sb[:, :, hh * 65 + 64: hh * 65 + 65], 1.0)
                    for r in range(ng):
                        for half in range(2):
                            c0 = r * 256 + half * 128
                            nc.scalar.dma_start(out=k_sb[:, :, c0:c0 + 128],
                                                in_=g_kv[r, :, half])
                            nc.scalar.dma_start(out=v4[:, 2 * r + half, :, 0:64],
                                                in_=g_kv[r, :, 2 + half])

                    # attention: every core runs 16 heads x 8 kv tiles over its
                    # own 256 q tokens; causality is the maskat multiply.
                    o_own = act.tile([128, ET, OWN], bf, tag="oown", name="o_own")
                    pending = []

                    def emit_normalize_kv(job):
                        jph, jet, jpo = job
                        dinv = small.tile([1, 256], f32, tag="dinv", name="dinv")
                        nc.vector.reciprocal(dinv[:], jpo[64:65, :])
                        bc = psm.tile([64, 256], f32, space="PSUM", tag="psm", name="bc")
                        nc.tensor.matmul(bc[:], lhsT=ones_row[:, :64], rhs=dinv[:],
                                         start=True, stop=True)
                        binv = small.tile([64, 256], f32, tag="binv", name="binv")
                        nc.vector.tensor_copy(out=binv[:], in_=bc[:])
                        nc.vector.tensor_mul(out=o_own[jph:jph + 64, jet, :],
                                             in0=jpo[0:64, :], in1=binv[:])

                    for hh in range(16):
                        et_h, ph = hh // 2, (hh % 2) * 64
                        po_t = pso.tile([65, 256], f32, space="PSUM", tag="pso",
                                        name="po_t")
                        for pr in range(4):
                            pst = pss.tile([128, 512], f32, space="PSUM", tag="pss",
                                           name="s_ps")
                            for sub in range(2):
                                t = 2 * pr + sub
                                nc.tensor.matmul(
                                    pst[:, sub * 256:(sub + 1) * 256],
                                    lhsT=k_sb[ph:ph + 64, et_h, t * 128:(t + 1) * 128],
                                    rhs=q_sb[ph:ph + 64, et_h, :],
                                    start=True, stop=True)
                            pt = small.tile([128, 512], bf, tag="pt", bufs=ptb, name="pt")
                            nc.scalar.activation(pt[:], pst[:], AF.Exp)
                            nc.vector.tensor_mul(out=pt[:], in0=pt[:],
                                                 in1=maskat[:, pr * 512:(pr + 1) * 512])
                            for sub in range(2):
                                t = 2 * pr + sub
                                nc.tensor.matmul(
                                    po_t[:], lhsT=v_sb[:, t, hh * 65: hh * 65 + 65],
                                    rhs=pt[:, sub * 256:(sub + 1) * 256],
                                    start=(t == 0), stop=(t == 7))
                        pending.append((ph, et_h, po_t))
                        if len(pending) > 1:
                            emit_normalize_kv(pending.pop(0))
                    while pending:
                        emit_normalize_kv(pending.pop(0))

                    # Wo is fully local now (no partial-sum ReduceScatter):
                    # x += Wo^T o + bo, with LN2 stats emitted inline.
                    wo_sb = wts.tile([128, ET, E], bf, tag="wo", bufs=1, name="wo_sb")
                    nc.sync.dma_start(out=wo_sb[:], in_=wo_d[l])
                    bo_sb = small.tile([128, ET], f32, tag="lng", name="bo_sb")
                    nc.sync.dma_start(out=bo_sb[:], in_=bo_d[l])
                    ln2_xx2 = []
                    for eo in range(ET):
                        ps = pmm.tile([128, OWN], f32, space="PSUM", tag="pmm",
                                      name="wo_ps")
                        for et in range(ET):
                            nc.tensor.matmul(
                                ps[:], lhsT=wo_sb[:, et, eo * 128:(eo + 1) * 128],
                                rhs=o_own[:, et, :], start=(et == 0), stop=(et == ET - 1))
                        nc.vector.scalar_tensor_tensor(
                            out=x[:, eo, :], in0=ps[:], scalar=bo_sb[:, eo:eo + 1],
                            in1=x[:, eo, :], op0=OP.add, op1=OP.add)
                        ln2_xx2.append(ln_stats_emit(x[:, eo, :]))

                if scheme == "head":
                    # collective-output loads go on the ACT queue so they don't
                    # head-of-line-block weight streaming on the sync queue.
                    # Token-major layout [128, et, 1024]: the QKV matmul rhs
                    # slices are then CONTIGUOUS 512-column runs (a moving
                    # operand with a free-dim discontinuity costs ~20ns extra
                    # per matmul on HW).
                    gh_sb = act.tile([128, ET, ng * OWN], bf, tag="gh", name="gh_sb")
                    for q in range(4):
                        for r in range(ng):
                            nc.scalar.dma_start(
                                out=gh_sb[:, 2 * q:2 * q + 2, r * OWN:(r + 1) * OWN],
                                in_=g_hq[q][r])

                    # QKV for this core's 4 heads over the whole batch (1024 tok)
                    wqkv_sb = wts.tile([128, ET, 768], bf, tag="wqkv", bufs=2, name="wqkv_sb")
                    nc.sync.dma_start(out=wqkv_sb[:], in_=wqkv_d[l])
                    # hoist the first two W1 chunk loads behind wqkv on the
                    # sync queue: their buffers freed when the previous
                    # layer's FFN finished, so both stream in during the
                    # attention phase instead of stalling FFN startup
                    for hc in range(2):
                        w1_c = wts.tile([128, ET, 1024], bf, tag="wchunk", name="w1_c")
                        nc.sync.dma_start(out=w1_c[:], in_=w1_d[l, hc])
                        w1_pre.append(w1_c)
                    q_sb = act.tile([128, 2, 1024], bf, tag="q", name="q_sb")
                    k_sb = act.tile([128, 2, 1024], bf, tag="k", name="k_sb")
                    v_dm = act.tile([128, 2, 1024], bf, tag="vdm", name="v_dm")
                    dsts = [q_sb, k_sb, v_dm]
                    # V first: its PE transposes then overlap the Q/K matmuls
                    for wi in (2, 0, 1):
                        for ct in range(2):
                            for chk in range(2):
                                ps = pmm.tile([128, 512], f32, space="PSUM", tag="pmm", name="qkv_ps")
                                for et in range(ET):
                                    nc.tensor.matmul(
                                        ps[:], lhsT=wqkv_sb[:, et, wi * 256 + ct * 128: wi * 256 + (ct + 1) * 128],
                                        rhs=gh_sb[:, et, chk * 512:(chk + 1) * 512],
                                        start=(et == 0), stop=(et == ET - 1))
                                # psum->SBUF eviction on the (idle) scalar engine,
                                # freeing DVE for the V-layout copies
                                nc.scalar.activation(dsts[wi][:, ct, chk * 512:(chk + 1) * 512],
                                                     ps[:], AF.Copy)

                    # V -> token-major with a ones column per head (65 cols/head)
                    v_sb = act.tile([128, 8, 260], bf, tag="vtk", name="v_sb")
                    for hh in range(4):
                        nc.gpsimd.memset(v_sb[:, :, hh * 65 + 64: hh * 65 + 65], 1.0)
                    for ct in range(2):
                        for tt in range(8):
                            pst = pss.tile([128, 128], bf, space="PSUM", tag="pss", name="vt_ps")
                            nc.tensor.transpose(pst[:], v_dm[:, ct, tt * 128:(tt + 1) * 128], ident[:])
                            for sub in range(2):
                                hh = ct * 2 + sub
                                nc.vector.tensor_copy(
                                    out=v_sb[:, tt, hh * 65: hh * 65 + 64],
                                    in_=pst[:, sub * 64:(sub + 1) * 64])

                    # attention: 4 heads x 4 q-block pairs, causal triangle.
                    # Normalization is fully off the PE: fast approx
                    # reciprocal (single custom-DVE op) + DMA partition-
                    # broadcast + DVE multiply, emitted inline per head.
                    o_own = act.tile([128, 2, 1024], bf, tag="oown", name="o_own")

                    def emit_normalize(jpb, jct, jp, jpo):
                        dinv = small.tile([1, 256], f32, tag="dinv", name="dinv")
                        # (reciprocal_approx_fast mis-evaluates through this
                        # build path — custom-DVE table never takes effect —
                        # so keep the exact InstReciprocal)
                        nc.vector.reciprocal(dinv[:], jpo[64:65, :])
                        binv = small.tile([64, 256], f32, tag="binv", name="binv")
                        nc.gpsimd.partition_broadcast(binv[:], dinv[:])
                        nc.vector.tensor_mul(
                            out=o_own[jpb:jpb + 64, jct, jp * 256:(jp + 1) * 256],
                            in0=jpo[0:64, :], in1=binv[:])

                    # kv tiles processed in pairs sharing one [128,512] PSUM bank:
                    # halves the Exp count (ACT is the attention-phase bottleneck)
                    # and the diagonal pair takes one fused [128,512] mask multiply.
                    # q-blocks are OUTER and each block's summed projection is
                    # produced by its OWN 4-way AllReduce, launched as soon as
                    # that block's Wo partial is stored — blocks 0..2 fly while
                    # later blocks' attention still runs, and only block 3's
                    # (et-split) AllReduce tail is exposed.  Each core then
                    # selects its own block out of the four AllReduce results
                    # with a 0/1 per-core mask (SPMD: rank-dependence lives in
                    # mask DATA, the program is identical).  bo/ng is folded
                    # into the partial eviction so the residual needs no
                    # separate bias pass.
                    wo_sb = wts.tile([128, 2, 1024], bf, tag="wo", bufs=2, name="wo_sb")
                    nc.sync.dma_start(out=wo_sb[:], in_=wo_d[l])
                    bo_sb = small.tile([128, ET], f32, tag="lng", name="bo_sb")
                    nc.sync.dma_start(out=bo_sb[:], in_=bo_d[l])
                    part_sb = act.tile([128, ng, ET, OWN], bf, tag="osb", name="part_sb")
                    part_d = dram.tile([ng, 128, ET, OWN], bf, tag="part_d", name="part_d")
                    EH = ET // 2
                    part3_d = [dram.tile([128, EH, OWN], bf, tag=f"part3_d{hf}",
                                         name=f"part3_d{hf}") for hf in range(2)]
                    ar_o = [dram.tile([128, ET, OWN], bf, tag=f"ar_o{p}",
                                      addr_space=maybe_share_collective_output_space(
                                          "AllReduce", grp4),
                                      name=f"ar_o{p}") for p in range(3)]
                    ar3_o = [dram.tile([128, EH, OWN], bf, tag=f"ar3_o{hf}",
                                       addr_space=maybe_share_collective_output_space(
                                           "AllReduce", grp4),
                                       name=f"ar3_o{hf}") for hf in range(2)]

                    def launch_ar(in_ap, out_ap):
                        if no_cc or no_rs:
                            nc.sync.dma_start(out=out_ap, in_=in_ap)
                        else:
                            nc.gpsimd.collective_compute(
                                "AllReduce", OP.add, replica_groups=grp4,
                                ins=[in_ap.opt()], outs=[out_ap.opt()])

                    def emit_wo_partial(r):
                        for eo in range(ET):
                            ps = pmm.tile([128, OWN], f32, space="PSUM", tag="pmm", name="wo_ps")
                            for ci in range(2):
                                nc.tensor.matmul(ps[:], lhsT=wo_sb[:, ci, eo * 128:(eo + 1) * 128],
                                                 rhs=o_own[:, ci, r * 256:(r + 1) * 256],
                                                 start=(ci == 0), stop=(ci == 1))
                            nc.scalar.activation(part_sb[:, r, eo, :], ps[:], AF.Copy)
                            if r == 3 and eo == EH - 1:
                                nc.gpsimd.dma_start(out=part3_d[0][:],
                                                    in_=part_sb[:, 3, 0:EH, :])
                                launch_ar(part3_d[0][:], ar3_o[0][:])
                        if r < 3:
                            nc.gpsimd.dma_start(out=part_d[r], in_=part_sb[:, r, :, :])
                            launch_ar(part_d[r], ar_o[r][:])
                        else:
                            nc.gpsimd.dma_start(out=part3_d[1][:],
                                                in_=part_sb[:, 3, EH:ET, :])
                            launch_ar(part3_d[1][:], ar3_o[1][:])

                    for p in range(4):
                        nkv = 2 * p + 2
                        for hh in range(4):
                            pb = (hh % 2) * 64
                            ct = hh // 2
                            po_t = pso.tile([65, 256], f32, space="PSUM", tag="pso", name="po_t")
                            for pr in range(p + 1):
                                pst = pss.tile([128, 512], f32, space="PSUM", tag="pss", name="s_ps")
                                for sub in range(2):
                                    t = 2 * pr + sub
                                    nc.tensor.matmul(
                                        pst[:, sub * 256:(sub + 1) * 256],
                                        lhsT=k_sb[pb:pb + 64, ct, t * 128:(t + 1) * 128],
                                        rhs=q_sb[pb:pb + 64, ct, p * 256:(p + 1) * 256],
                                        start=True, stop=True)
                                pt = small.tile([128, 512], bf, tag="pt", bufs=ptb, name="pt")
                                nc.scalar.activation(pt[:], pst[:], AF.Exp)
                                if pr == p:  # diagonal pair
                                    nc.vector.tensor_mul(out=pt[:], in0=pt[:], in1=maskd[:])
                                for sub in range(2):
                                    t = 2 * pr + sub
                                    nc.tensor.matmul(
                                        po_t[:], lhsT=v_sb[:, t, hh * 65: hh * 65 + 65],
                                        rhs=pt[:, sub * 256:(sub + 1) * 256],
                                        start=(t == 0), stop=(t == nkv - 1))
                            emit_normalize(pb, ct, p, po_t)
                            if p >= 1 and hh == 0:
                                # block p-1 is fully normalized; its Wo
                                # partial + AllReduce launch overlap the rest
                                # of block p's attention
                                emit_wo_partial(p - 1)
                    emit_wo_partial(3)

                    # reuses part_sb's buffer (same tag/shape): the last read
                    # of part_sb is block 3's store to DRAM, which precedes
                    # every AllReduce result landing anyway
                    ars = act.tile([128, ng, ET, OWN], bf, tag="osb", name="ars")
                    for p2 in range(3):
                        nc.scalar.dma_start(out=ars[:, p2, :, :], in_=ar_o[p2][:])
                    nc.scalar.dma_start(out=ars[:, 3, 0:EH, :], in_=ar3_o[0][:])
                    nc.scalar.dma_start(out=ars[:, 3, EH:ET, :], in_=ar3_o[1][:])

                    ln2_xx2 = []
                    for eo in range(ET):
                        nc.vector.tensor_scalar_add(x[:, eo, :], x[:, eo, :],
                                                    bo_sb[:, eo:eo + 1])
                        for p2 in range(ng):
                            nc.vector.scalar_tensor_tensor(
                                out=x[:, eo, :], in0=ars[:, p2, eo, :],
                                scalar=grpmask[:, p2:p2 + 1],
                                in1=x[:, eo, :], op0=OP.mult, op1=OP.add)
                        ln2_xx2.append(ln_stats_emit(x[:, eo, :]))

                # FFN
                ln2g = small.tile([128, ET], f32, tag="lng", name="ln2g")
                ln2b = small.tile([128, ET], f32, tag="lnb", name="ln2b")
                nc.sync.dma_start(out=ln2g[:], in_=ln2g_d[l])
                nc.sync.dma_start(out=ln2b[:], in_=ln2b_d[l])
                h2_sb = act.tile([128, ET, OWN], bf, tag="h", name="h2_sb")
                s_ps2 = ln_stats_reduce(ln2_xx2)
                ln_finish(s_ps2, ln2g[:], ln2b[:], h2_sb[:], x[:])

                b1_sb = small.tile([128, FT], f32, tag="b1", name="b1_sb")
                nc.sync.dma_start(out=b1_sb[:], in_=b1_d[l])
                g_ffn = act.tile([128, FT, OWN], bf, tag="big", name="g_ffn")

                def w2_chunk(ec):
                    c = wts.tile([128, FT, 256], bf, tag="wchunk", name="w2_c")
                    nc.sync.dma_start(out=c[:], in_=w2_d[l, ec])
                    return c

                w2_pre = None
                for hc in range(4):
                    if hc < len(w1_pre):
                        w1_c = w1_pre[hc]
                    else:
                        w1_c = wts.tile([128, ET, 1024], bf, tag="wchunk", name="w1_c")
                        nc.sync.dma_start(out=w1_c[:], in_=w1_d[l, hc])
                    if hc == 3:
                        # W2 chunk 0's buffer freed once W1 chunk 1 was
                        # consumed; its load overlaps the last W1 chunk
                        w2_pre = w2_chunk(0)
                    for ho in range(8):
                        ps = pmm.tile([128, OWN], f32, space="PSUM", tag="pmm", name="w1_ps")
                        for et in range(ET):
                            nc.tensor.matmul(ps[:], lhsT=w1_c[:, et, ho * 128:(ho + 1) * 128],
                                             rhs=h2_sb[:, et, :], start=(et == 0), stop=(et == ET - 1))
                        hidx = hc * 8 + ho
                        gelu_af = AF.Gelu if gelu_kind == "gelu" else AF.Identity
                        nc.scalar.activation(g_ffn[:, hidx, :], ps[:], gelu_af,
                                             bias=b1_sb[:, hidx:hidx + 1])
                w1_pre = []

                b2_sb = small.tile([128, ET], f32, tag="lnb", name="b2_sb")
                nc.sync.dma_start(out=b2_sb[:], in_=b2_d[l])
                ln1_xx2 = []
                for ec in range(4):
                    w2_c = w2_pre if ec == 0 else w2_chunk(ec)
                    for eo2 in range(2):
                        eo = ec * 2 + eo2
                        ps = pmm.tile([128, OWN], f32, space="PSUM", tag="pmm", name="w2_ps")
                        for ht in range(FT):
                            nc.tensor.matmul(ps[:], lhsT=w2_c[:, ht, eo2 * 128:(eo2 + 1) * 128],
                                             rhs=g_ffn[:, ht, :], start=(ht == 0), stop=(ht == FT - 1))
                        nc.vector.scalar_tensor_tensor(
                            out=x[:, eo, :], in0=ps[:], scalar=b2_sb[:, eo:eo + 1],
                            in1=x[:, eo, :], op0=OP.add, op1=OP.add)
                        # stats for the next LN (layer l+1's LN1 / final LN)
                        ln1_xx2.append(ln_stats_emit(x[:, eo, :]))

            # ---- final LN + AllGather + lm_head -----------------------
            hf_sb = act.tile([128, ET, OWN], bf, tag="h", name="hf_sb")
            s_psf = ln_stats_reduce(ln1_xx2)
            # first two lm_head weight chunks stream in during the final LN
            # and the 8-way AllGather (their buffers freed by the last FFN)
            wlm_pre = []
            for vc in range(2):
                wlm_c = wts.tile([128, ET, vt_pc * 128], bf, tag="wchunk", name="wlm_c")
                nc.sync.dma_start(out=wlm_c[:], in_=wlm_d[vc])
                wlm_pre.append(wlm_c)
            # final gather split into 4 et-pair chunks like the per-layer
            # AllGather, launched from inside the LN apply; lm_head (which
            # consumes et in order) starts as soon as the first chunk lands.
            hf_shard = dram.tile([4, 128, 2, OWN], bf, tag="h_shard", name="hf_shard")
            g_hfq = [dram.tile([n_cores, 128, 2, OWN], bf, tag=f"g_hfq{q}",
                               addr_space="Local" if no_cc else
                               maybe_share_collective_output_space("AllGather", grp8),
                               name=f"g_hfq{q}") for q in range(4)]

            def ag_f_cb(et):
                if et % 2 == 1:
                    q = et // 2
                    nc.gpsimd.dma_start(out=hf_shard[q], in_=hf_sb[:, et - 1:et + 1, :])
                    if no_cc:
                        for r in range(n_cores):
                            nc.sync.dma_start(out=g_hfq[q][r], in_=hf_shard[q])
                    else:
                        nc.gpsimd.collective_compute(
                            "AllGather", OP.bypass, replica_groups=grp8,
                            ins=[hf_shard[q][:].opt()], outs=[g_hfq[q][:].opt()])

            ln_finish(s_psf, lnfg[:], lnfb[:], hf_sb[:], x[:], chunk_cb=ag_f_cb)
            # token-major gathered layout: lm_head rhs slices are contiguous
            ghf_sb = act.tile([128, ET, n_cores * OWN], bf, tag="big", name="ghf_sb")
            for q in range(4):
                for r in range(n_cores):
                    nc.scalar.dma_start(
                        out=ghf_sb[:, 2 * q:2 * q + 2, r * OWN:(r + 1) * OWN],
                        in_=g_hfq[q][r])

            NT = n_cores * OWN  # 2048 token columns
            for vc in range(n_chunks):
                if vc < len(wlm_pre):
                    wlm_c = wlm_pre[vc]
                else:
                    wlm_c = wts.tile([128, ET, vt_pc * 128], bf, tag="wchunk", name="wlm_c")
                    nc.sync.dma_start(out=wlm_c[:], in_=wlm_d[vc])
                for vt in range(vt_pc):
                    vidx = vc * vt_pc + vt
                    for tc_ in range(NT // 512):
                        ps = pmm.tile([128, 512], f32, space="PSUM", tag="pmm", name="lm_ps")
                        for et in range(ET):
                            nc.tensor.matmul(
                                ps[:], lhsT=wlm_c[:, et, vt * 128:(vt + 1) * 128],
                                rhs=ghf_sb[:, et, tc_ * 512:(tc_ + 1) * 512],
                                start=(et == 0), stop=(et == ET - 1))
                        lg = small.tile([128, 512], f32, tag="lg", bufs=lgb, name="lg")
                        nc.scalar.activation(lg[:], ps[:], AF.Identity, bias=blm[:, vidx:vidx + 1])
                        nc.gpsimd.dma_start(out=logits_d[vidx, tc_], in_=lg[:])

    nc.compile()
    return nc


def _pmajor2(a, tiles):
    """[N] -> [128, tiles] with element (p, t) = a[t*128+p]."""
    return np.ascontiguousarray(a.reshape(tiles, 128).T)


def _pmajor3(a, tiles):
    """[N, C] -> [128, tiles, C] with (p, t, c) = a[t*128+p, c]."""
    n, c = a.shape
    return np.ascontiguousarray(a.reshape(tiles, 128, c).transpose(1, 0, 2))


def prep_in_maps(inputs, cfg=None):
    """Slice/cast/lay out the full inputs into per-core input maps."""
    cfg = cfg or _full_cfg()
    nL, nVPC, n_cores = cfg["L"], cfg["VPC"], cfg["n_cores"]
    scheme = cfg.get("scheme", "head")
    t1b, ptb, lgb = (4, 4, 3) if scheme == "head" else (2, 2, 2)
    nVT = nVPC // 128
    n_chunks = max(1, nVT // 5)
    vt_pc = nVT // n_chunks
    f = lambda a: np.asarray(a, dtype=np.float32)
    idx = np.asarray(inputs["idx"])
    tok_emb = f(inputs["tok_emb"])
    pos_emb = f(inputs["pos_emb"])
    Wq, Wk, Wv = f(inputs["Wq"]), f(inputs["Wk"]), f(inputs["Wv"])
    Wo = f(inputs["Wo"])
    W1, W2 = f(inputs["W1"]), f(inputs["W2"])
    Wlm = f(inputs["Wlm"])

    # replicated weights, host-laid-out once
    w1_h = np.ascontiguousarray(
        W1[:nL].reshape(nL, ET, 128, 4, 1024).transpose(0, 3, 2, 1, 4)).astype(BF16)
    w2_h = np.ascontiguousarray(
        W2[:nL].reshape(nL, FT, 128, 4, 256).transpose(0, 3, 2, 1, 4)).astype(BF16)
    ln_h = {k: np.stack([_pmajor2(f(inputs[k])[l], ET) for l in range(nL)])
            for k in ("ln1_g", "ln1_b", "ln2_g", "ln2_b", "bo", "b2")}
    b1_h = np.stack([_pmajor2(f(inputs["b1"])[l], FT) for l in range(nL)])
    lnfg_h = _pmajor2(f(inputs["lnf_g"]), ET)
    lnfb_h = _pmajor2(f(inputs["lnf_b"]), ET)

    wlm_pad = np.zeros((E, nVPC * n_cores), dtype=np.float32)
    nv = min(V, nVPC * n_cores)
    wlm_pad[:, :nv] = Wlm[:, :nv]
    blm_pad = np.zeros((nVPC * n_cores,), dtype=np.float32)
    blm_pad[:nv] = f(inputs["blm"])[:nv]

    kp = np.arange(128)[:, None]
    qf = np.arange(256)[None, :]
    masklo = np.where(qf < 128, kp <= qf, True).astype(BF16)
    maskhi = np.where(qf < 128, False, kp <= qf - 128).astype(BF16)
    maskd = np.concatenate([masklo, maskhi], axis=1)  # [128, 512]

    if scheme == "kv":
        # replicated full QKV/Wo weights, host-laid-out once
        stack = np.stack([Wk[:nL], Wv[:nL], Wq[:nL] * (1.0 / np.sqrt(HD))], axis=1)
        wqkv_kv = np.ascontiguousarray(
            stack.reshape(nL, 3, ET, 128, E).transpose(0, 1, 3, 2, 4)).astype(BF16)
        wo_kv = np.ascontiguousarray(
            Wo[:nL].reshape(nL, ET, 128, E).transpose(0, 2, 1, 3)).astype(BF16)

    maps = []
    for c in range(n_cores):
        b, g = (c // 4, c % 4) if n_cores == 8 else (0, 0)
        tloc = g * OWN + np.arange(OWN)
        rows = idx[b, tloc].astype(np.int64)
        if scheme == "kv":
            wqkv_h, wo_h = wqkv_kv, wo_kv
        else:
            qs = slice(g * 256, (g + 1) * 256)
            # fold the 1/sqrt(HD) attention scale into Wq on the host
            wqkv = np.concatenate(
                [Wq[:nL, :, qs] * (1.0 / np.sqrt(HD)), Wk[:nL, :, qs], Wv[:nL, :, qs]],
                axis=2)
            wqkv_h = np.ascontiguousarray(
                wqkv.reshape(nL, ET, 128, 768).transpose(0, 2, 1, 3)).astype(BF16)
            wo_h = np.ascontiguousarray(
                Wo[:nL, qs, :].reshape(nL, 2, 128, E).transpose(0, 2, 1, 3)).astype(BF16)
        wlm_slice = wlm_pad[:, c * nVPC:(c + 1) * nVPC]
        wlm_h = np.ascontiguousarray(
            wlm_slice.reshape(ET, 128, n_chunks, vt_pc * 128).transpose(2, 1, 0, 3)).astype(BF16)
        m = {
            "x0": _pmajor3(tok_emb[rows].T.copy().reshape(E, OWN), ET).copy(),
            "pose": _pmajor3(pos_emb[tloc].T.copy().reshape(E, OWN), ET).copy(),
            "wqkv": wqkv_h, "wo": wo_h, "w1": w1_h, "w2": w2_h, "wlm": wlm_h,
            "ln1g": ln_h["ln1_g"], "ln1b": ln_h["ln1_b"],
            "ln2g": ln_h["ln2_g"], "ln2b": ln_h["ln2_b"],
            "bo": ln_h["bo"], "b1": b1_h, "b2": ln_h["b2"],
            "lnfg": lnfg_h, "lnfb": lnfb_h,
            "blm": _pmajor2(blm_pad[c * nVPC:(c + 1) * nVPC], nVT),
        }
        if scheme == "kv":
            # visibility of kv tile t from own q position j: kv <= q global
            m["maskat"] = np.concatenate(
                [((t * 128 + kp) <= (g * 256 + qf)) for t in range(8)],
                axis=1).astype(BF16)
        else:
            m["maskd"] = maskd
            ngg = 4 if n_cores == 8 else n_cores
            gm = np.zeros((128, ngg), dtype=np.float32)
            gm[:, g % ngg] = 1.0
            m["grpmask"] = gm
        maps.append(m)
    return maps


def make_runner(nc, n_cores=NCORES):
    """Build a reusable jitted SPMD executor (mirrors bass2jax.run_bass_via_pjrt
    multi-core path, without donation so it can be re-invoked for timing)."""
    import jax
    from jax.experimental.shard_map import shard_map
    from jax.sharding import Mesh, PartitionSpec, NamedSharding
    from concourse import bass2jax, mybir

    bass2jax.install_neuronx_cc_hook()
    partition_name = nc.partition_id_tensor.name if nc.partition_id_tensor else None
    in_names, out_names, out_avals = [], [], []
    for alloc in nc.m.functions[0].allocations:
        if not isinstance(alloc, mybir.MemoryLocationSet):
            continue
        name = alloc.memorylocations[0].name
        if alloc.kind == "ExternalInput":
            if name != partition_name:
                in_names.append(name)
        elif alloc.kind == "ExternalOutput":
            assert alloc.tensor_shape is not None
            out_names.append(name)
            out_avals.append(jax.core.ShapedArray(
                tuple(alloc.tensor_shape), mybir.dt.np(alloc.dtype)))
    n_params, n_outs = len(in_names), len(out_names)
    all_in = list(in_names) + list(out_names)
    if partition_name:
        all_in.append(partition_name)

    def _body(*args):
        operands = list(args)
        if partition_name:
            operands.append(bass2jax.partition_id_tensor())
        outs = bass2jax._bass_exec_p.bind(
            *operands, out_avals=tuple(out_avals), in_names=tuple(all_in),
            out_names=tuple(out_names), lowering_input_output_aliases=(),
            sim_require_finite=True, sim_require_nnan=True, nc=nc)
        return tuple(outs)

    devices = jax.devices()[:n_cores]
    mesh = Mesh(np.asarray(devices), ("core",))
    sharded = jax.jit(
        shard_map(_body, mesh=mesh,
                  in_specs=(PartitionSpec("core"),) * (n_params + n_outs),
                  out_specs=(PartitionSpec("core"),) * n_outs,
                  check_rep=False),
        keep_unused=True)
    sharding = NamedSharding(mesh, PartitionSpec("core"))
    return sharded, in_names, out_names, out_avals, sharding


def run(nc, in_maps, n_cores=NCORES, time_iters=0, batch_iters=None):
    """Execute; returns (results_per_core, timing_dict_or_None).

    The output of the first execution is cross-checked against a second
    execution: the collective firmware very occasionally returns stale
    data (observed ~1/20 runs), so a mismatch triggers a retry and the
    majority result wins.
    """
    import jax, time
    sharded, in_names, out_names, out_avals, sharding = make_runner(nc, n_cores)
    concat_in = [np.concatenate([np.asarray(m[nm]) for m in in_maps], axis=0)
                 for nm in in_names]
    concat_zero = [np.zeros((n_cores * a.shape[0], *a.shape[1:]), a.dtype)
                   for a in out_avals]
    args = [jax.device_put(a, sharding) for a in (*concat_in, *concat_zero)]

    def exec_once():
        out = sharded(*args)
        jax.block_until_ready(out)
        return [np.asarray(o) for o in out]

    out = exec_once()
    if nc.has_collectives:
        out2 = exec_once()
        agree = all(np.array_equal(a, b) for a, b in zip(out, out2))
        if not agree:
            out3 = exec_once()
            if all(np.array_equal(a, b) for a, b in zip(out2, out3)):
                out = out2
            else:
                out = out3

    best = None
    if time_iters:
        serial = None
        for _ in range(time_iters):
            t0 = time.perf_counter()
            o = sharded(*args)
            jax.block_until_ready(o)
            dt_ = time.perf_counter() - t0
            serial = dt_ if serial is None else min(serial, dt_)
        batches = []
        n_batches, batch = batch_iters or (6, 8)
        for _ in range(n_batches):
            t0 = time.perf_counter()
            outs = [sharded(*args) for _ in range(batch)]
            jax.block_until_ready(outs)
            batches.append((time.perf_counter() - t0) / batch)
        best = {"serial": serial, "pipelined": min(batches), "batches": batches}

    results = [
        {nm: out[i].reshape(n_cores, *out_avals[i].shape)[c]
         for i, nm in enumerate(out_names)}
        for c in range(n_cores)
    ]
    return results, best


def assemble_logits(results, cfg=None):
    """[nVT, NT/512, 128, 512] tile-major per-core outputs -> [B, T, Vtot]."""
    cfg = cfg or _full_cfg()
    n_cores, nVPC = cfg["n_cores"], cfg["VPC"]
    per_core = [results[c]["logits"].transpose(0, 2, 1, 3).reshape(nVPC, B * T)
                for c in range(n_cores)]
    full = np.concatenate(per_core, axis=0)  # [Vpad, B*T]
    return full


def kernel(**inputs) -> np.ndarray:
    if "nc" not in _CACHE:
        _CACHE["nc"] = build_program()
    nc = _CACHE["nc"]
    in_maps = prep_in_maps(inputs)
    results, _ = run(nc, in_maps, NCORES, time_iters=0)
    _CACHE["last_results"] = results
    full = assemble_logits(results)
    return np.ascontiguousarray(full[:V].T).reshape(B, T, V)

